# revision 1
# baseline (speedup 1.0000x reference)
"""MoE layer (8 experts, top-2) on 8 TRN2 NeuronCores, expert-parallel.

Strategy (sparse dispatch, per the sharding hint):
  - Core m owns expert m (w1[m], w2[m], b1[m], b2[m]).
  - Host computes top-2 expert ids per token (fp32 router, dispatch only)
    and "all-to-all"s: each core receives only the tokens routed to its
    expert, gathered as X_c^T [H, C] (C = max expert load, rounded to 128).
  - On device, each core re-runs the router (fp32 matmul on PE) over its
    gathered tokens and derives ITS OWN expert's combine weight per token
    purely elementwise:
        w_e(t) = exp(l_e - m1) / (1 + exp(m2 - m1))  if l_e >= m2 else 0
    (equals softmax-top2-renormalize of the reference).
  - FFN in bf16 (f32 PSUM accumulate): h1 = gelu(x @ w1 + b1) in [F, C]
    layout; y = (h1^T @ w2 + b2) * w with tokens on partitions -> yc [C, H].
  - Host scatter-adds each core's weighted outputs back to token order.
"""

from contextlib import ExitStack

import ml_dtypes
import numpy as np

P = 128
B, S, H, F, E = 2, 2048, 1024, 4096, 8
T = B * S            # 4096 tokens
KH = H // P          # 8   k-subtiles over H
KF = F // P          # 32  k-subtiles over F

_CACHE = {}


def _chunks(C):
    out = []
    t0 = 0
    while t0 < C:
        size = min(512, C - t0)
        out.append((t0, size))
        t0 += size
    return out


def _build_nc(C, reps=1):
    import concourse.mybir as mybir
    import concourse.tile as tile
    from concourse import bacc

    dt = mybir.dt
    AF = mybir.ActivationFunctionType
    ALU = mybir.AluOpType
    AX = mybir.AxisListType

    TTc = C // P  # token tiles

    nc = bacc.Bacc(
        "TRN2", target_bir_lowering=False, debug=False, num_devices=E)

    xct32 = nc.declare_dram_parameter("xct32", [H, C], dt.float32, isOutput=False)
    xctb = nc.declare_dram_parameter("xctb", [H, C], dt.bfloat16, isOutput=False)
    rw = nc.declare_dram_parameter("rw", [H, E], dt.float32, isOutput=False)
    rbb = nc.declare_dram_parameter("rbb", [P, E], dt.float32, isOutput=False)
    selb = nc.declare_dram_parameter("selb", [P, E], dt.float32, isOutput=False)
    w1d = nc.declare_dram_parameter("w1d", [H, F], dt.bfloat16, isOutput=False)
    w2d = nc.declare_dram_parameter("w2d", [F, H], dt.bfloat16, isOutput=False)
    b1d = nc.declare_dram_parameter("b1d", [P, KF], dt.float32, isOutput=False)
    b2b = nc.declare_dram_parameter("b2b", [P, H], dt.float32, isOutput=False)
    yc = nc.declare_dram_parameter("yc", [C, H], dt.float32, isOutput=True)

    xct32_r = xct32.rearrange("(k p) t -> p k t", p=P)
    xctb_r = xctb.rearrange("(k p) t -> p k t", p=P)
    rw_r = rw.rearrange("(k p) e -> p k e", p=P)
    w1_r = w1d.rearrange("(k p) f -> p k f", p=P)
    w2_r = w2d.rearrange("(k p) h -> p k h", p=P)

    with ExitStack() as ctx:
        tc = ctx.enter_context(tile.TileContext(nc))
        const = ctx.enter_context(tc.tile_pool(name="const", bufs=1))
        xrpool = ctx.enter_context(tc.tile_pool(name="xr", bufs=2))
        rpool = ctx.enter_context(tc.tile_pool(name="rtmp", bufs=3))
        rpsum = ctx.enter_context(tc.tile_pool(name="rpsum", bufs=1, space="PSUM"))
        xpool = ctx.enter_context(tc.tile_pool(name="xc", bufs=2))
        h1pool = ctx.enter_context(tc.tile_pool(name="h1", bufs=1))
        p1pool = ctx.enter_context(tc.tile_pool(name="p1", bufs=5, space="PSUM"))
        p2pool = ctx.enter_context(tc.tile_pool(name="p2", bufs=2, space="PSUM"))
        opool = ctx.enter_context(tc.tile_pool(name="ob", bufs=8))

        # Small constants first so nothing queues behind the weight stacks.
        # (b2b is 0.5MB and not needed until the first output stage ~70us in,
        # so it loads after the weight stream instead.)
        rbb_s = const.tile([P, E], dt.float32)
        nc.sync.dma_start(rbb_s[:], rbb[:])
        selb_s = const.tile([P, E], dt.float32)
        nc.sync.dma_start(selb_s[:], selb[:])
        b1_s = const.tile([P, KF], dt.float32)
        nc.sync.dma_start(b1_s[:], b1d[:])
        rw_s = const.tile([P, KH, E], dt.float32)
        b2b_s = const.tile([P, H], dt.float32)
        wmat = const.tile([P, TTc], dt.float32)

        chunks = _chunks(C)

        def load_xc(t0, csz):
            xc = xpool.tile([P, KH, 512], dt.bfloat16, name="xc")[:, :, :csz]
            for k in range(KH):
                nc.sync.dma_start(xc[:, k], xctb_r[:, k, t0:t0 + csz])
            return xc

        # Startup: interleave chunk-0 activations with w1's first f-chunk
        # per k so the first matmul group is runnable after ~2MB of DMA.
        # Then w1 f-chunk-major with w2 k-slices interleaved at a ratio
        # that keeps DMA just ahead of PE's w1 consumption, so w2 is
        # resident before chunk-0 matmul2 starts (~70us in).
        w1_s = const.tile([P, KH, F], dt.bfloat16)
        w2_s = const.tile([P, KF, H], dt.bfloat16)
        xc0 = xpool.tile([P, KH, 512], dt.bfloat16, name="xc")[:, :, :chunks[0][1]]
        for k in range(KH):
            nc.sync.dma_start(xc0[:, k], xctb_r[:, k, 0:chunks[0][1]])
            nc.sync.dma_start(w1_s[:, k, 0:512], w1_r[:, k, 0:512])
        w2_next = 0
        for fc in range(1, F // 512):
            for k in range(KH):
                nc.sync.dma_start(
                    w1_s[:, k, fc * 512:(fc + 1) * 512],
                    w1_r[:, k, fc * 512:(fc + 1) * 512])
            share = 0 if fc < 2 else (5 if fc < 7 else KF - w2_next)
            for k in range(w2_next, w2_next + share):
                nc.sync.dma_start(w2_s[:, k], w2_r[:, k])
            w2_next += share
            if fc == 4:
                nc.sync.dma_start(rw_s[:], rw_r)
        nc.sync.dma_start(b2b_s[:], b2b[:])

        def emit_mm1(xc, csz):
            h1 = h1pool.tile([P, KF, 512], dt.bfloat16, name="h1")[:, :, :csz]
            for f in range(KF):
                ps1 = p1pool.tile([P, 512], dt.float32, name="ps1")[:, :csz]
                for k in range(KH):
                    nc.tensor.matmul(
                        ps1[:], w1_s[:, k, f * P:(f + 1) * P], xc[:, k],
                        start=(k == 0), stop=(k == KH - 1),
                    )
                nc.scalar.activation(h1[:, f], ps1[:], AF.Gelu, bias=b1_s[:, f:f + 1])
            return h1

        def emit_mm2(h1, t0, csz, tail_split=False):
            for ct in range(csz // P):
                gt = t0 // P + ct
                for hh in range(H // 512):
                    last = tail_split and ct == csz // P - 1 and hh == H // 512 - 1
                    # The very last group splits in two halves so its output
                    # pipeline (DVE + DMA) overlaps the second half's matmuls
                    # instead of running serially after PE finishes.
                    for (o0, wid) in ([(0, 256), (256, 128), (384, 64), (448, 64)] if last else [(0, 512)]):
                        ps2 = p2pool.tile([P, 512], dt.float32, name="ps2")[:, :wid]
                        for k in range(KF):
                            nc.tensor.matmul(
                                ps2[:], h1[:, k, ct * P:(ct + 1) * P],
                                w2_s[:, k, hh * 512 + o0:hh * 512 + o0 + wid],
                                start=(k == 0), stop=(k == KF - 1),
                            )
                        ob = opool.tile([P, 512], dt.float32, name="ob")[:, :wid]
                        nc.vector.tensor_tensor(
                            ob[:], ps2[:],
                            b2b_s[:, hh * 512 + o0:hh * 512 + o0 + wid], ALU.add)
                        nc.vector.tensor_scalar_mul(ob[:], ob[:], wmat[:, gt:gt + 1])
                        nc.sync.dma_start(
                            yc[gt * P:(gt + 1) * P,
                               hh * 512 + o0:hh * 512 + o0 + wid], ob[:])

        for _rep in range(reps):
            # Chunk-0 first FFN matmul overlaps the router's DMAs.
            h1_0 = emit_mm1(xc0, chunks[0][1])

            # ---- Router: combine weight of MY expert for my gathered tokens ----
            for tt in range(TTc):
                xt_t = xrpool.tile([P, KH, P], dt.float32)
                nc.sync.dma_start(xt_t[:], xct32_r[:, :, tt * P:(tt + 1) * P])
                lg = rpsum.tile([P, E], dt.float32)
                for k in range(KH):
                    nc.tensor.matmul(
                        lg[:], xt_t[:, k], rw_s[:, k],
                        start=(k == 0), stop=(k == KH - 1),
                    )
                l = rpool.tile([P, E], dt.float32)
                nc.vector.tensor_tensor(l[:], lg[:], rbb_s[:], ALU.add)
                m1 = rpool.tile([P, 1], dt.float32)
                nc.vector.reduce_max(m1[:], l[:], axis=AX.X)
                nm1 = rpool.tile([P, 1], dt.float32)
                nc.vector.tensor_scalar_mul(nm1[:], m1[:], -1.0)
                ismax = rpool.tile([P, E], dt.float32)
                nc.vector.tensor_tensor(
                    ismax[:], l[:], m1[:].to_broadcast((P, E)), ALU.is_equal)
                pen = rpool.tile([P, E], dt.float32)
                nc.vector.tensor_scalar_mul(pen[:], ismax[:], 1e30)
                lmask = rpool.tile([P, E], dt.float32)
                nc.vector.tensor_tensor(lmask[:], l[:], pen[:], ALU.subtract)
                m2 = rpool.tile([P, 1], dt.float32)
                nc.vector.reduce_max(m2[:], lmask[:], axis=AX.X)
                lsel = rpool.tile([P, E], dt.float32)
                nc.vector.tensor_tensor(lsel[:], l[:], selb_s[:], ALU.mult)
                lmine = rpool.tile([P, 1], dt.float32)
                nc.vector.reduce_sum(lmine[:], lsel[:], axis=AX.X)
                ge = rpool.tile([P, 1], dt.float32)
                nc.vector.tensor_tensor(ge[:], lmine[:], m2[:], ALU.is_ge)
                e1 = rpool.tile([P, 1], dt.float32)
                nc.scalar.activation(e1[:], lmine[:], AF.Exp, bias=nm1[:])
                e2 = rpool.tile([P, 1], dt.float32)
                nc.scalar.activation(e2[:], m2[:], AF.Exp, bias=nm1[:])
                den = rpool.tile([P, 1], dt.float32)
                nc.vector.tensor_scalar_add(den[:], e2[:], 1.0)
                rec = rpool.tile([P, 1], dt.float32)
                nc.vector.reciprocal(rec[:], den[:])
                wnum = rpool.tile([P, 1], dt.float32)
                nc.vector.tensor_tensor(wnum[:], e1[:], ge[:], ALU.mult)
                nc.vector.tensor_tensor(wmat[:, tt:tt + 1], wnum[:], rec[:], ALU.mult)

            # ---- Expert FFN over gathered tokens, weighted output ----
            emit_mm2(h1_0, chunks[0][0], chunks[0][1],
                     tail_split=(len(chunks) == 1))
            for ci, (t0, csz) in enumerate(chunks[1:], start=1):
                xc = load_xc(t0, csz)
                h1 = emit_mm1(xc, csz)
                emit_mm2(h1, t0, csz, tail_split=(ci == len(chunks) - 1))
    return nc


def _get_nc(C, reps=1):
    key = (C, reps)
    if key not in _CACHE:
        nc = _build_nc(C, reps)
        nc.finalize()
        _CACHE[key] = nc
    return _CACHE[key]


def dispatch(hidden_states, router_w, router_b):
    """Host-side top-2 dispatch: per-expert token index lists + capacity."""
    x = np.asarray(hidden_states, dtype=np.float32).reshape(T, H)
    logits = x @ np.asarray(router_w, dtype=np.float32)
    logits = logits + np.asarray(router_b, dtype=np.float32)
    top2 = np.argpartition(logits, E - 2, axis=1)[:, E - 2:]  # [T, 2] unordered
    idx_lists = []
    for m in range(E):
        idx_lists.append(np.where((top2 == m).any(axis=1))[0])
    cmax = max(len(ix) for ix in idx_lists)
    C = max(P, ((cmax + P - 1) // P) * P)
    return x, idx_lists, C


def make_in_maps(hidden_states, router_w, router_b, w1, b1, w2, b2):
    bf16 = ml_dtypes.bfloat16
    x, idx_lists, C = dispatch(hidden_states, router_w, router_b)
    xt = np.ascontiguousarray(x.T)            # [H, T] f32
    xtb = xt.astype(bf16)
    rw = np.ascontiguousarray(np.asarray(router_w, dtype=np.float32))
    rbb = np.ascontiguousarray(
        np.broadcast_to(np.asarray(router_b, dtype=np.float32), (P, E)))
    w1 = np.asarray(w1, dtype=np.float32)
    w2 = np.asarray(w2, dtype=np.float32)
    b1 = np.asarray(b1, dtype=np.float32)
    b2 = np.asarray(b2, dtype=np.float32)
    in_maps = []
    for m in range(E):
        ix = idx_lists[m]
        pad = np.zeros(C, dtype=np.int64)
        pad[:len(ix)] = ix
        sel = np.zeros((P, E), dtype=np.float32)
        sel[:, m] = 1.0
        in_maps.append({
            "xct32": np.ascontiguousarray(xt[:, pad]),
            "xctb": np.ascontiguousarray(xtb[:, pad]),
            "rw": rw,
            "rbb": rbb,
            "selb": sel,
            "w1d": np.ascontiguousarray(w1[m].astype(bf16)),
            "w2d": np.ascontiguousarray(w2[m].astype(bf16)),
            "b1d": np.ascontiguousarray(b1[m].reshape(KF, P).T),
            "b2b": np.ascontiguousarray(np.broadcast_to(b2[m], (P, H))),
        })
    return in_maps, idx_lists, C


def run_device(in_maps, C):
    from concourse.bass_utils import run_bass_kernel_spmd

    nc = _get_nc(C)
    res = run_bass_kernel_spmd(nc, in_maps, core_ids=list(range(E)))
    return res.results


def kernel(hidden_states, router_w, router_b, w1, b1, w2, b2):
    in_maps, idx_lists, C = make_in_maps(
        hidden_states, router_w, router_b, w1, b1, w2, b2)
    # One retry guards against a rare transient execution glitch observed on
    # the very first load of a freshly compiled NEFF (garbage ~1e35 values);
    # a healthy output has absmax of a few units.
    last_err = None
    for attempt in range(3):
        try:
            results = run_device(in_maps, C)
        except Exception as e:  # transient NRT/axon failures observed
            last_err = e
            import time as _time
            _time.sleep(10)
            continue
        acc = np.zeros((T, H), dtype=np.float32)
        for m in range(E):
            ix = idx_lists[m]
            acc[ix] += np.asarray(results[m]["yc"], dtype=np.float32)[:len(ix)]
        if np.isfinite(acc).all() and np.abs(acc).max() < 1e4:
            return acc.reshape(B, S, H)
    if last_err is not None:
        raise last_err
    return acc.reshape(B, S, H)



# revision 5
# speedup vs baseline: 1.0766x; 1.0766x over previous
"""MoE layer (8 experts, top-2) on 8 TRN2 NeuronCores, expert-parallel.

Strategy (sparse dispatch + mixed-precision mm2):
  - Host computes the router (fp32 logits, top-2, softmax renormalize) and
    dispatches: core m owns expert m's weights.
  - Per expert, tokens sort by combine weight. The G=768 highest-weight
    token-expert pairs run fully in bf16. The remainder (low combine
    weight) runs mm1 in bf16 but mm2 in fp8 e4m3 with DoubleRow perf mode
    (2x PE throughput); the fp8 quantization error is suppressed by those
    tokens' small combine weights (overall rel-err ~1.4e-2 < 2e-2).
  - SPMD static shapes per core: G bf16 tokens + 256-token fp8 slot A
    (own expert) + 64-token fp8 slot B (another expert's overflow, that
    expert's weights are DMA'd to this core). Underfilled slots pad with
    zero-weight tokens.
  - Combine weights apply on device (DVE); b2 is added on host (it only
    multiplies combine weights, which the host has).
"""

from contextlib import ExitStack

import ml_dtypes
import numpy as np

P = 128
B, S, H, F, E = 2, 2048, 1024, 4096, 8
T = B * S            # 4096 tokens
KH = H // P          # 8   k-subtiles over H
KF = F // P          # 32  k-subtiles over F

G_DEF = 768          # bf16 tokens per core (multiple of 128)
FA_DEF = 256         # fp8 slot A capacity (own expert, multiple of 64)
FB_DEF = 64          # fp8 slot B capacity (spill expert, multiple of 64)

bf16 = ml_dtypes.bfloat16
f8 = ml_dtypes.float8_e4m3fn
WSCALE = 64.0        # fp8 w2 pre-scale (folded into combine weights)

_CACHE = {}


def _build_nc(G, FA, FB):
    import concourse.mybir as mybir
    import concourse.tile as tile
    from concourse import bacc

    dt = mybir.dt
    AF = mybir.ActivationFunctionType
    DR = mybir.MatmulPerfMode.DoubleRow

    GT = G // P                    # bf16 token tiles (6)
    NT = G + FA + FB               # tokens per core (1088)
    NAT = FA // 64                 # fp8 A mm2 tiles (4)
    NBT = FB // 64                 # fp8 B mm2 tiles (1)

    nc = bacc.Bacc(
        "TRN2", target_bir_lowering=False, debug=False, num_devices=E)

    xb = nc.declare_dram_parameter("xb", [H, NT], dt.bfloat16, isOutput=False)
    w1b = nc.declare_dram_parameter("w1b", [H, F], dt.bfloat16, isOutput=False)
    w2b = nc.declare_dram_parameter("w2b", [F, H], dt.bfloat16, isOutput=False)
    w1s = nc.declare_dram_parameter("w1s", [H, F], dt.bfloat16, isOutput=False)
    w2a = nc.declare_dram_parameter("w2a", [F, H], dt.float8e4, isOutput=False)
    w2s = nc.declare_dram_parameter("w2s", [F, H], dt.float8e4, isOutput=False)
    b1o = nc.declare_dram_parameter("b1o", [P, KF], dt.float32, isOutput=False)
    b1s = nc.declare_dram_parameter("b1s", [P, KF], dt.float32, isOutput=False)
    wmb = nc.declare_dram_parameter("wmb", [P, GT], dt.float32, isOutput=False)
    wma = nc.declare_dram_parameter("wma", [64, NAT], dt.float32, isOutput=False)
    wms = nc.declare_dram_parameter("wms", [64, NBT], dt.float32, isOutput=False)
    yc = nc.declare_dram_parameter("yc", [NT, H], dt.float32, isOutput=True)

    xb_r = xb.rearrange("(k p) t -> p k t", p=P)
    w1b_r = w1b.rearrange("(k p) f -> p k f", p=P)
    w2b_r = w2b.rearrange("(k p) h -> p k h", p=P)
    w1s_r = w1s.rearrange("(k p) f -> p k f", p=P)
    w2a_r = w2a.rearrange("(k p) h -> p k h", p=P)
    w2s_r = w2s.rearrange("(k p) h -> p k h", p=P)

    with ExitStack() as ctx:
        tc = ctx.enter_context(tile.TileContext(nc))
        const = ctx.enter_context(tc.tile_pool(name="const", bufs=1))
        wpool = ctx.enter_context(tc.tile_pool(name="w", bufs=4))
        xbp = ctx.enter_context(tc.tile_pool(name="xb", bufs=2))
        h1bp = ctx.enter_context(tc.tile_pool(name="h1b", bufs=1))
        h1fp = ctx.enter_context(tc.tile_pool(name="h1f", bufs=2))
        opool = ctx.enter_context(tc.tile_pool(name="ob", bufs=7))
        p1 = ctx.enter_context(tc.tile_pool(name="p1", bufs=5, space="PSUM"))
        p2 = ctx.enter_context(tc.tile_pool(name="p2", bufs=3, space="PSUM"))

        # ---- small constants ----
        b1o_s = const.tile([P, KF], dt.float32)
        nc.sync.dma_start(b1o_s[:], b1o[:])
        b1s_s = const.tile([P, KF], dt.float32)
        nc.sync.dma_start(b1s_s[:], b1s[:])
        wmb_s = const.tile([P, GT], dt.float32)
        nc.sync.dma_start(wmb_s[:], wmb[:])
        wma_s = const.tile([64, NAT], dt.float32)
        nc.sync.dma_start(wma_s[:], wma[:])
        wms_s = const.tile([64, NBT], dt.float32)
        nc.sync.dma_start(wms_s[:], wms[:])

        # ---- x chunk loads ----
        def load_x(t0, csz):
            xt = xbp.tile([P, KH, 512], dt.bfloat16, name="xbt")[:, :, :csz]
            for k in range(KH):
                nc.sync.dma_start(xt[:, k], xb_r[:, k, t0:t0 + csz])
            return xt

        xc0 = load_x(0, 512)

        # ---- bf16 weights (slots 0..3), f/h-chunked for early PE start ----
        w1b_t = []
        for half in range(2):
            t = wpool.tile([P, KH, F // 2], dt.bfloat16, name="w")
            w1b_t.append(t)
            for fc in range(4):
                nc.sync.dma_start(
                    t[:, :, fc * 512:(fc + 1) * 512],
                    w1b_r[:, :, half * (F // 2) + fc * 512:half * (F // 2) + (fc + 1) * 512])
        w2b_t = []
        for half in range(2):
            t = wpool.tile([P, KF, H // 2], dt.bfloat16, name="w")
            w2b_t.append(t)
            for hc in range(2):
                nc.sync.dma_start(
                    t[:, :, hc * 256:(hc + 1) * 256],
                    w2b_r[:, :, half * (H // 2) + hc * 256:half * (H // 2) + (hc + 1) * 256])

        # ---- late weights; slot reuse orders their DMA firing ----
        # w2a -> slot0 (w1b half1: dead mid-cA-mm1), w2s -> slot1 (w1b
        # half2: dead at cA-mm1 end), w1s halves -> slots 2,3 (w2b: dead
        # by c1-mm2 end, well before cB-mm1 needs them).
        w2a_t = wpool.tile([P, KF, H], dt.float8e4, name="w")
        w2s_t = wpool.tile([P, KF, H], dt.float8e4, name="w")
        w1s_t = [wpool.tile([P, KH, F // 2], dt.bfloat16, name="w")
                 for _ in range(2)]
        for half in range(2):
            for fc in range(4):
                nc.sync.dma_start(
                    w1s_t[half][:, :, fc * 512:(fc + 1) * 512],
                    w1s_r[:, :, half * (F // 2) + fc * 512:half * (F // 2) + (fc + 1) * 512])
        for hc in range(2):
            nc.sync.dma_start(w2a_t[:, :, hc * 512:(hc + 1) * 512],
                              w2a_r[:, :, hc * 512:(hc + 1) * 512])
        for hc in range(2):
            nc.sync.dma_start(w2s_t[:, :, hc * 512:(hc + 1) * 512],
                              w2s_r[:, :, hc * 512:(hc + 1) * 512])

        # ---- mm1 (bf16) ----
        def mm1(xt, csz, w1t, b1t, h1, hdt):
            for ft in range(KF):
                src = w1t[ft // 16]
                ps = p1.tile([P, 512], dt.float32, name="ps1")[:, :csz]
                for k in range(KH):
                    nc.tensor.matmul(
                        ps[:], src[:, k, (ft % 16) * P:(ft % 16 + 1) * P], xt[:, k],
                        start=(k == 0), stop=(k == KH - 1))
                nc.scalar.activation(h1[:, ft], ps[:], AF.Gelu,
                                     bias=b1t[:, ft:ft + 1])

        # ---- mm2 bf16 ----
        def mm2_bf16(h1, t0, csz):
            for ct in range(csz // P):
                gt = t0 // P + ct
                for hh in range(2):
                    src = w2b_t[hh]
                    ps2 = p2.tile([P, 512], dt.float32, name="ps2")
                    for kf in range(KF):
                        nc.tensor.matmul(
                            ps2[:], h1[:, kf, ct * P:(ct + 1) * P], src[:, kf, :],
                            start=(kf == 0), stop=(kf == KF - 1))
                    ob = opool.tile([P, 512], dt.float32, name="ob")
                    nc.vector.tensor_scalar_mul(ob[:], ps2[:], wmb_s[:, gt:gt + 1])
                    nc.sync.dma_start(
                        yc[gt * P:(gt + 1) * P, hh * 512:(hh + 1) * 512], ob[:])

        # ---- mm2 fp8 (DoubleRow) ----
        def mm2_fp8(h1, w2t, wmt, t0, csz):
            for tt in range(csz // 64):
                for hh in range(4):
                    ps2 = p2.tile([P, 512], dt.float32, name="ps2")[:64, :256]
                    for kp in range(KF // 2):
                        nc.tensor.matmul(
                            ps2[:], h1[:, 2 * kp:2 * kp + 2, tt * 64:(tt + 1) * 64],
                            w2t[:, 2 * kp:2 * kp + 2, hh * 256:(hh + 1) * 256],
                            start=(kp == 0), stop=(kp == KF // 2 - 1),
                            perf_mode=DR)
                    ob = opool.tile([P, 512], dt.float32, name="ob")[:64, :256]
                    nc.vector.tensor_scalar_mul(ob[:], ps2[:], wmt[:, tt:tt + 1])
                    nc.sync.dma_start(
                        yc[t0 + tt * 64:t0 + (tt + 1) * 64,
                           hh * 256:(hh + 1) * 256], ob[:])

        # ---- schedule ----
        h1 = h1bp.tile([P, KF, 512], dt.bfloat16, name="h1b")
        mm1(xc0, 512, w1b_t, b1o_s, h1, dt.bfloat16)
        mm2_bf16(h1, 0, 512)

        xc1 = load_x(512, 256)
        h1 = h1bp.tile([P, KF, 512], dt.bfloat16, name="h1b")[:, :, :256]
        mm1(xc1, 256, w1b_t, b1o_s, h1, dt.bfloat16)
        mm2_bf16(h1, 512, 256)

        xca = load_x(G, FA)
        h1a = h1fp.tile([P, KF, FA], dt.float8e4, name="h1f")
        mm1(xca, FA, w1b_t, b1o_s, h1a, dt.float8e4)
        mm2_fp8(h1a, w2a_t, wma_s, G, FA)

        xcb = load_x(G + FA, FB)
        h1b2 = h1fp.tile([P, KF, FA], dt.float8e4, name="h1f")[:, :, :FB]
        mm1(xcb, FB, w1s_t, b1s_s, h1b2, dt.float8e4)
        mm2_fp8(h1b2, w2s_t, wms_s, G + FA, FB)
    return nc


def _get_nc(G, FA, FB):
    key = (G, FA, FB)
    if key not in _CACHE:
        nc = _build_nc(G, FA, FB)
        nc.finalize()
        _CACHE[key] = nc
    return _CACHE[key]


def dispatch(hidden_states, router_w, router_b):
    """Host router: top-2 ids + renormalized combine weights per token."""
    x = np.asarray(hidden_states, dtype=np.float32).reshape(T, H)
    logits = x @ np.asarray(router_w, dtype=np.float32)
    logits = logits + np.asarray(router_b, dtype=np.float32)
    part = np.argpartition(logits, E - 2, axis=1)[:, E - 2:]      # [T,2] unordered
    pv = np.take_along_axis(logits, part, axis=1)
    swap = pv[:, 0] > pv[:, 1]
    i1 = np.where(swap, part[:, 0], part[:, 1])
    i2 = np.where(swap, part[:, 1], part[:, 0])
    l1 = logits[np.arange(T), i1]
    l2 = logits[np.arange(T), i2]
    e2 = np.exp((l2 - l1).astype(np.float64))
    wt1 = (1.0 / (1.0 + e2)).astype(np.float32)
    wt2 = (e2 / (1.0 + e2)).astype(np.float32)
    return x, i1, i2, wt1, wt2


def plan(i1, i2, wt1, wt2, G, FA, FB):
    """Token->(core, group) assignment."""
    bf_tok, bf_wt, a_tok, a_wt, spill = [], [], [], [], []
    for m in range(E):
        tk = np.concatenate([np.where(i1 == m)[0], np.where(i2 == m)[0]])
        wt = np.concatenate([wt1[i1 == m], wt2[i2 == m]])
        o = np.argsort(-wt)
        tk, wt = tk[o], wt[o]
        bf_tok.append(tk[:G])
        bf_wt.append(wt[:G])
        a_tok.append(tk[G:G + FA])
        a_wt.append(wt[G:G + FA])
        rest_t, rest_w = tk[G + FA:], wt[G + FA:]
        for s0 in range(0, len(rest_t), FB):
            spill.append((m, rest_t[s0:s0 + FB], rest_w[s0:s0 + FB]))
    assert len(spill) <= E, f"spill slots {len(spill)} > {E}"
    while len(spill) < E:
        spill.append((0, np.zeros(0, np.int64), np.zeros(0, np.float32)))
    return bf_tok, bf_wt, a_tok, a_wt, spill


def make_in_maps(hidden_states, router_w, router_b, w1, b1, w2, b2,
                 G=G_DEF, FA=FA_DEF, FB=FB_DEF):
    x, i1, i2, wt1, wt2 = dispatch(hidden_states, router_w, router_b)
    bf_tok, bf_wt, a_tok, a_wt, spill = plan(i1, i2, wt1, wt2, G, FA, FB)
    w1 = np.asarray(w1, dtype=np.float32)
    w2 = np.asarray(w2, dtype=np.float32)
    b1 = np.asarray(b1, dtype=np.float32)
    b2 = np.asarray(b2, dtype=np.float32)
    xt = np.ascontiguousarray(x.T)                      # [H, T]
    GT = G // P

    def wcol(wts, cap, rows, scale):
        ncol = cap // rows
        out = np.zeros((rows, ncol), dtype=np.float32)
        wv = np.zeros(cap, dtype=np.float32)
        wv[:len(wts)] = wts * scale
        for c in range(ncol):
            out[:, c] = wv[c * rows:(c + 1) * rows]
        return out

    w1b16 = [np.ascontiguousarray(w1[m].astype(bf16)) for m in range(E)]
    w2f8 = [np.ascontiguousarray((w2[m] * WSCALE).astype(f8)) for m in range(E)]
    b1r = [np.ascontiguousarray(b1[m].reshape(KF, P).T) for m in range(E)]

    in_maps = []
    for m in range(E):
        sm, st, sw = spill[m]
        xbm = np.zeros((H, G + FA + FB), dtype=bf16)
        xbm[:, :len(bf_tok[m])] = xt[:, bf_tok[m]].astype(bf16)
        xbm[:, G:G + len(a_tok[m])] = xt[:, a_tok[m]].astype(bf16)
        xbm[:, G + FA:G + FA + len(st)] = xt[:, st].astype(bf16)
        in_maps.append({
            "xb": xbm,
            "w1b": w1b16[m],
            "w2b": np.ascontiguousarray(w2[m].astype(bf16)),
            "w1s": w1b16[sm],
            "w2a": w2f8[m], "w2s": w2f8[sm],
            "b1o": b1r[m], "b1s": b1r[sm],
            "wmb": wcol(bf_wt[m], GT * P, P, 1.0),
            "wma": wcol(a_wt[m], FA, 64, 1.0 / WSCALE),
            "wms": wcol(sw, FB, 64, 1.0 / WSCALE),
        })
    meta = (bf_tok, a_tok, spill, i1, i2, wt1, wt2)
    return in_maps, meta


def run_device(in_maps, G=G_DEF, FA=FA_DEF, FB=FB_DEF):
    from concourse.bass_utils import run_bass_kernel_spmd

    nc = _get_nc(G, FA, FB)
    res = run_bass_kernel_spmd(nc, in_maps, core_ids=list(range(E)))
    return res.results


def kernel(hidden_states, router_w, router_b, w1, b1, w2, b2):
    G, FA, FB = G_DEF, FA_DEF, FB_DEF
    in_maps, meta = make_in_maps(
        hidden_states, router_w, router_b, w1, b1, w2, b2, G, FA, FB)
    bf_tok, a_tok, spill, i1, i2, wt1, wt2 = meta
    b2 = np.asarray(b2, dtype=np.float32)
    # One retry guards against rare transient NRT/axon failures.
    last_err = None
    for attempt in range(3):
        try:
            results = run_device(in_maps, G, FA, FB)
        except Exception as e:
            last_err = e
            import time as _time
            _time.sleep(10)
            continue
        acc = np.zeros((T, H), dtype=np.float32)
        for m in range(E):
            ycm = np.asarray(results[m]["yc"], dtype=np.float32)
            acc[bf_tok[m]] += ycm[:len(bf_tok[m])]
            if len(a_tok[m]):
                acc[a_tok[m]] += ycm[G:G + len(a_tok[m])]
            sm, st, sw = spill[m]
            if len(st):
                acc[st] += ycm[G + FA:G + FA + len(st)]
        # b2 contribution (combine-weighted), host-side
        acc += wt1[:, None] * b2[i1] + wt2[:, None] * b2[i2]
        if np.isfinite(acc).all() and np.abs(acc).max() < 1e4:
            return acc.reshape(B, S, H)
    if last_err is not None:
        raise last_err
    return acc.reshape(B, S, H)


# revision 10
# speedup vs baseline: 1.0958x; 1.0177x over previous
"""MoE layer (8 experts, top-2) on 8 TRN2 NeuronCores, expert-parallel.

Strategy (sparse dispatch + mixed-precision mm2):
  - Host computes the router (fp32 logits, top-2, softmax renormalize) and
    dispatches: core m owns expert m's weights.
  - Per expert, tokens sort by combine weight. The G=768 highest-weight
    token-expert pairs run fully in bf16. The remainder (low combine
    weight) runs mm1 in bf16 but mm2 in fp8 e4m3 with DoubleRow perf mode
    (2x PE throughput); the fp8 quantization error is suppressed by those
    tokens' small combine weights (overall rel-err ~1.4e-2 < 2e-2).
  - SPMD static shapes per core: G bf16 tokens + 256-token fp8 slot A
    (own expert) + 64-token fp8 slot B (another expert's overflow, that
    expert's weights are DMA'd to this core). Underfilled slots pad with
    zero-weight tokens.
  - Combine weights apply on device (DVE); b2 is added on host (it only
    multiplies combine weights, which the host has).
"""

from contextlib import ExitStack

import ml_dtypes
import numpy as np

P = 128
B, S, H, F, E = 2, 2048, 1024, 4096, 8
T = B * S            # 4096 tokens
KH = H // P          # 8   k-subtiles over H
KF = F // P          # 32  k-subtiles over F

G_DEF = 768          # bf16 tokens per core (multiple of 128)
FA_DEF = 256         # fp8 slot A capacity (own expert, multiple of 64)
FB_DEF = 64          # fp8 slot B capacity (spill expert, multiple of 64)

bf16 = ml_dtypes.bfloat16
f8 = ml_dtypes.float8_e4m3fn
WSCALE = 64.0        # fp8 w2 pre-scale (folded into combine weights)

_CACHE = {}


def _build_nc(G, FA, FB):
    import concourse.mybir as mybir
    import concourse.tile as tile
    from concourse import bacc

    dt = mybir.dt
    AF = mybir.ActivationFunctionType
    DR = mybir.MatmulPerfMode.DoubleRow

    GT = G // P                    # bf16 token tiles (6)
    NT = G + FA + FB               # tokens per core (1088)
    NAT = FA // 64                 # fp8 A mm2 tiles (4)
    NBT = FB // 64                 # fp8 B mm2 tiles (1)

    nc = bacc.Bacc(
        "TRN2", target_bir_lowering=False, debug=False, num_devices=E)

    xb = nc.declare_dram_parameter("xb", [H, NT], dt.bfloat16, isOutput=False)
    w1b = nc.declare_dram_parameter("w1b", [H, F], dt.bfloat16, isOutput=False)
    w2b = nc.declare_dram_parameter("w2b", [F, H], dt.bfloat16, isOutput=False)
    w1s = nc.declare_dram_parameter("w1s", [H, F], dt.bfloat16, isOutput=False)
    w2a = nc.declare_dram_parameter("w2a", [F, H], dt.float8e4, isOutput=False)
    w2s = nc.declare_dram_parameter("w2s", [F, H], dt.float8e4, isOutput=False)
    b1o = nc.declare_dram_parameter("b1o", [P, KF], dt.float32, isOutput=False)
    b1s = nc.declare_dram_parameter("b1s", [P, KF], dt.float32, isOutput=False)
    wmb = nc.declare_dram_parameter("wmb", [P, GT], dt.float32, isOutput=False)
    wma = nc.declare_dram_parameter("wma", [64, NAT], dt.float32, isOutput=False)
    wms = nc.declare_dram_parameter("wms", [64, NBT], dt.float32, isOutput=False)
    yc = nc.declare_dram_parameter("yc", [NT, H], dt.float32, isOutput=True)

    xb_r = xb.rearrange("(k p) t -> p k t", p=P)
    w1b_r = w1b.rearrange("(k p) f -> p k f", p=P)
    w2b_r = w2b.rearrange("(k p) h -> p k h", p=P)
    w1s_r = w1s.rearrange("(k p) f -> p k f", p=P)
    w2a_r = w2a.rearrange("(k p) h -> p k h", p=P)
    w2s_r = w2s.rearrange("(k p) h -> p k h", p=P)

    with ExitStack() as ctx:
        tc = ctx.enter_context(tile.TileContext(nc))
        const = ctx.enter_context(tc.tile_pool(name="const", bufs=1))
        w1pool = ctx.enter_context(tc.tile_pool(name="w1", bufs=8))
        w2pool = ctx.enter_context(tc.tile_pool(name="w2", bufs=2))
        xbp = ctx.enter_context(tc.tile_pool(name="xb", bufs=2))
        h1bp = ctx.enter_context(tc.tile_pool(name="h1b", bufs=1))
        h1fp = ctx.enter_context(tc.tile_pool(name="h1f", bufs=2))
        opool = ctx.enter_context(tc.tile_pool(name="ob", bufs=7))
        p1 = ctx.enter_context(tc.tile_pool(name="p1", bufs=5, space="PSUM"))
        p2 = ctx.enter_context(tc.tile_pool(name="p2", bufs=3, space="PSUM"))

        # ---- small constants ----
        b1o_s = const.tile([P, KF], dt.float32)
        nc.sync.dma_start(b1o_s[:], b1o[:])
        b1s_s = const.tile([P, KF], dt.float32)
        nc.sync.dma_start(b1s_s[:], b1s[:])
        wmb_s = const.tile([P, GT], dt.float32)
        nc.sync.dma_start(wmb_s[:], wmb[:])
        wma_s = const.tile([64, NAT], dt.float32)
        nc.sync.dma_start(wma_s[:], wma[:])
        wms_s = const.tile([64, NBT], dt.float32)
        nc.sync.dma_start(wms_s[:], wms[:])

        # ---- x chunk loads ----
        def load_x(t0, csz):
            xt = xbp.tile([P, KH, 512], dt.bfloat16, name="xbt")[:, :, :csz]
            for k in range(KH):
                nc.sync.dma_start(xt[:, k], xb_r[:, k, t0:t0 + csz])
            return xt

        # Startup order: x chunk-0's k=0 plane, then w1b's first 512-f
        # tile, then the rest -- the first matmul is runnable after ~1.1MB.
        xc0 = xbp.tile([P, KH, 512], dt.bfloat16, name="xbt")
        nc.sync.dma_start(xc0[:, 0], xb_r[:, 0, 0:512])

        # w1 weights: 8 x 1MB tiles so matmuls march at 1MB granularity.
        w1b_q = []
        for q in range(8):
            t = w1pool.tile([P, KH, 512], dt.bfloat16, name="w1")
            w1b_q.append(t)
            nc.sync.dma_start(t[:], w1b_r[:, :, q * 512:(q + 1) * 512])
            if q == 0:
                for k in range(1, KH):
                    nc.sync.dma_start(xc0[:, k], xb_r[:, k, 0:512])
        w2b_t = []
        for half in range(2):
            t = w2pool.tile([P, KF, H // 2], dt.bfloat16, name="w2")
            w2b_t.append(t)
            for hc in range(2):
                nc.sync.dma_start(
                    t[:, :, hc * 256:(hc + 1) * 256],
                    w2b_r[:, :, half * (H // 2) + hc * 256:half * (H // 2) + (hc + 1) * 256])

        # ---- late weights; slot reuse + FIFO order hides their DMA ----
        # w2a reuses w2b half1's slot (dead ~27us before bf16 end), w1s
        # chunks reuse w1b's slots (dead progressively during cA-mm1),
        # w2s reuses w2b half2's slot; issue order matches need order.
        w2a_t = w2pool.tile([P, KF, H], dt.float8e4, name="w2")
        for hc in range(2):
            nc.sync.dma_start(w2a_t[:, :, hc * 512:(hc + 1) * 512],
                              w2a_r[:, :, hc * 512:(hc + 1) * 512])
        w1s_q = []
        for q in range(8):
            t = w1pool.tile([P, KH, 512], dt.bfloat16, name="w1")
            w1s_q.append(t)
            nc.sync.dma_start(t[:], w1s_r[:, :, q * 512:(q + 1) * 512])
        w2s_t = w2pool.tile([P, KF, H], dt.float8e4, name="w2")
        for hc in range(2):
            nc.sync.dma_start(w2s_t[:, :, hc * 512:(hc + 1) * 512],
                              w2s_r[:, :, hc * 512:(hc + 1) * 512])

        # ---- mm1 (bf16) ----
        def mm1(xt, csz, w1t, b1t, h1):
            for ft in range(KF):
                src = w1t[ft // 4]
                ps = p1.tile([P, 512], dt.float32, name="ps1")[:, :csz]
                for k in range(KH):
                    nc.tensor.matmul(
                        ps[:], src[:, k, (ft % 4) * P:(ft % 4 + 1) * P], xt[:, k],
                        start=(k == 0), stop=(k == KH - 1))
                nc.scalar.activation(h1[:, ft], ps[:], AF.Gelu,
                                     bias=b1t[:, ft:ft + 1])

        # ---- mm2 bf16 ----
        def mm2_bf16(h1, t0, csz):
            for ct in range(csz // P):
                gt = t0 // P + ct
                for hh in range(2):
                    src = w2b_t[hh]
                    ps2 = p2.tile([P, 512], dt.float32, name="ps2")
                    for kf in range(KF):
                        nc.tensor.matmul(
                            ps2[:], h1[:, kf, ct * P:(ct + 1) * P], src[:, kf, :],
                            start=(kf == 0), stop=(kf == KF - 1))
                    ob = opool.tile([P, 512], dt.float32, name="ob")
                    nc.vector.tensor_scalar_mul(ob[:], ps2[:], wmb_s[:, gt:gt + 1])
                    nc.sync.dma_start(
                        yc[gt * P:(gt + 1) * P, hh * 512:(hh + 1) * 512], ob[:])

        # ---- mm2 fp8 (DoubleRow) ----
        def mm2_fp8(h1, w2t, wmt, t0, csz):
            for tt in range(csz // 64):
                for hh in range(4):
                    ps2 = p2.tile([P, 512], dt.float32, name="ps2")[:64, :256]
                    for kp in range(KF // 2):
                        nc.tensor.matmul(
                            ps2[:], h1[:, 2 * kp:2 * kp + 2, tt * 64:(tt + 1) * 64],
                            w2t[:, 2 * kp:2 * kp + 2, hh * 256:(hh + 1) * 256],
                            start=(kp == 0), stop=(kp == KF // 2 - 1),
                            perf_mode=DR)
                    ob = opool.tile([P, 512], dt.float32, name="ob")[:64, :256]
                    nc.vector.tensor_scalar_mul(ob[:], ps2[:], wmt[:, tt:tt + 1])
                    nc.sync.dma_start(
                        yc[t0 + tt * 64:t0 + (tt + 1) * 64,
                           hh * 256:(hh + 1) * 256], ob[:])

        # ---- schedule ----
        h1 = h1bp.tile([P, KF, 512], dt.bfloat16, name="h1b")
        mm1(xc0, 512, w1b_q, b1o_s, h1)
        mm2_bf16(h1, 0, 512)

        xc1 = load_x(512, 256)
        h1 = h1bp.tile([P, KF, 512], dt.bfloat16, name="h1b")[:, :, :256]
        mm1(xc1, 256, w1b_q, b1o_s, h1)
        mm2_bf16(h1, 512, 256)

        xca = load_x(G, FA)
        h1a = h1fp.tile([P, KF, FA], dt.float8e4, name="h1f")
        mm1(xca, FA, w1b_q, b1o_s, h1a)
        mm2_fp8(h1a, w2a_t, wma_s, G, FA)

        xcb = load_x(G + FA, FB)
        h1b2 = h1fp.tile([P, KF, FA], dt.float8e4, name="h1f")[:, :, :FB]
        mm1(xcb, FB, w1s_q, b1s_s, h1b2)
        mm2_fp8(h1b2, w2s_t, wms_s, G + FA, FB)
    return nc


def _get_nc(G, FA, FB):
    key = (G, FA, FB)
    if key not in _CACHE:
        nc = _build_nc(G, FA, FB)
        nc.finalize()
        _CACHE[key] = nc
    return _CACHE[key]


def dispatch(hidden_states, router_w, router_b):
    """Host router: top-2 ids + renormalized combine weights per token."""
    x = np.asarray(hidden_states, dtype=np.float32).reshape(T, H)
    logits = x @ np.asarray(router_w, dtype=np.float32)
    logits = logits + np.asarray(router_b, dtype=np.float32)
    part = np.argpartition(logits, E - 2, axis=1)[:, E - 2:]      # [T,2] unordered
    pv = np.take_along_axis(logits, part, axis=1)
    swap = pv[:, 0] > pv[:, 1]
    i1 = np.where(swap, part[:, 0], part[:, 1])
    i2 = np.where(swap, part[:, 1], part[:, 0])
    l1 = logits[np.arange(T), i1]
    l2 = logits[np.arange(T), i2]
    e2 = np.exp((l2 - l1).astype(np.float64))
    wt1 = (1.0 / (1.0 + e2)).astype(np.float32)
    wt2 = (e2 / (1.0 + e2)).astype(np.float32)
    return x, i1, i2, wt1, wt2


def plan(i1, i2, wt1, wt2, G, FA, FB):
    """Token->(core, group) assignment."""
    bf_tok, bf_wt, a_tok, a_wt, spill = [], [], [], [], []
    for m in range(E):
        tk = np.concatenate([np.where(i1 == m)[0], np.where(i2 == m)[0]])
        wt = np.concatenate([wt1[i1 == m], wt2[i2 == m]])
        o = np.argsort(-wt)
        tk, wt = tk[o], wt[o]
        bf_tok.append(tk[:G])
        bf_wt.append(wt[:G])
        a_tok.append(tk[G:G + FA])
        a_wt.append(wt[G:G + FA])
        rest_t, rest_w = tk[G + FA:], wt[G + FA:]
        for s0 in range(0, len(rest_t), FB):
            spill.append((m, rest_t[s0:s0 + FB], rest_w[s0:s0 + FB]))
    assert len(spill) <= E, f"spill slots {len(spill)} > {E}"
    while len(spill) < E:
        spill.append((0, np.zeros(0, np.int64), np.zeros(0, np.float32)))
    return bf_tok, bf_wt, a_tok, a_wt, spill


def make_in_maps(hidden_states, router_w, router_b, w1, b1, w2, b2,
                 G=G_DEF, FA=FA_DEF, FB=FB_DEF):
    x, i1, i2, wt1, wt2 = dispatch(hidden_states, router_w, router_b)
    bf_tok, bf_wt, a_tok, a_wt, spill = plan(i1, i2, wt1, wt2, G, FA, FB)
    w1 = np.asarray(w1, dtype=np.float32)
    w2 = np.asarray(w2, dtype=np.float32)
    b1 = np.asarray(b1, dtype=np.float32)
    b2 = np.asarray(b2, dtype=np.float32)
    xt = np.ascontiguousarray(x.T)                      # [H, T]
    GT = G // P

    def wcol(wts, cap, rows, scale):
        ncol = cap // rows
        out = np.zeros((rows, ncol), dtype=np.float32)
        wv = np.zeros(cap, dtype=np.float32)
        wv[:len(wts)] = wts * scale
        for c in range(ncol):
            out[:, c] = wv[c * rows:(c + 1) * rows]
        return out

    w1b16 = [np.ascontiguousarray(w1[m].astype(bf16)) for m in range(E)]
    w2f8 = [np.ascontiguousarray((w2[m] * WSCALE).astype(f8)) for m in range(E)]
    b1r = [np.ascontiguousarray(b1[m].reshape(KF, P).T) for m in range(E)]

    in_maps = []
    for m in range(E):
        sm, st, sw = spill[m]
        xbm = np.zeros((H, G + FA + FB), dtype=bf16)
        xbm[:, :len(bf_tok[m])] = xt[:, bf_tok[m]].astype(bf16)
        xbm[:, G:G + len(a_tok[m])] = xt[:, a_tok[m]].astype(bf16)
        xbm[:, G + FA:G + FA + len(st)] = xt[:, st].astype(bf16)
        in_maps.append({
            "xb": xbm,
            "w1b": w1b16[m],
            "w2b": np.ascontiguousarray(w2[m].astype(bf16)),
            "w1s": w1b16[sm],
            "w2a": w2f8[m], "w2s": w2f8[sm],
            "b1o": b1r[m], "b1s": b1r[sm],
            "wmb": wcol(bf_wt[m], GT * P, P, 1.0),
            "wma": wcol(a_wt[m], FA, 64, 1.0 / WSCALE),
            "wms": wcol(sw, FB, 64, 1.0 / WSCALE),
        })
    meta = (bf_tok, a_tok, spill, i1, i2, wt1, wt2)
    return in_maps, meta


def run_device(in_maps, G=G_DEF, FA=FA_DEF, FB=FB_DEF):
    from concourse.bass_utils import run_bass_kernel_spmd

    nc = _get_nc(G, FA, FB)
    res = run_bass_kernel_spmd(nc, in_maps, core_ids=list(range(E)))
    return res.results


def kernel(hidden_states, router_w, router_b, w1, b1, w2, b2):
    G, FA, FB = G_DEF, FA_DEF, FB_DEF
    in_maps, meta = make_in_maps(
        hidden_states, router_w, router_b, w1, b1, w2, b2, G, FA, FB)
    bf_tok, a_tok, spill, i1, i2, wt1, wt2 = meta
    b2 = np.asarray(b2, dtype=np.float32)
    # One retry guards against rare transient NRT/axon failures.
    last_err = None
    for attempt in range(3):
        try:
            results = run_device(in_maps, G, FA, FB)
        except Exception as e:
            last_err = e
            import time as _time
            _time.sleep(10)
            continue
        acc = np.zeros((T, H), dtype=np.float32)
        for m in range(E):
            ycm = np.asarray(results[m]["yc"], dtype=np.float32)
            acc[bf_tok[m]] += ycm[:len(bf_tok[m])]
            if len(a_tok[m]):
                acc[a_tok[m]] += ycm[G:G + len(a_tok[m])]
            sm, st, sw = spill[m]
            if len(st):
                acc[st] += ycm[G + FA:G + FA + len(st)]
        # b2 contribution (combine-weighted), host-side
        acc += wt1[:, None] * b2[i1] + wt2[:, None] * b2[i2]
        if np.isfinite(acc).all() and np.abs(acc).max() < 1e4:
            return acc.reshape(B, S, H)
    if last_err is not None:
        raise last_err
    return acc.reshape(B, S, H)


# revision 15
# speedup vs baseline: 1.1366x; 1.0373x over previous
"""MoE layer (8 experts, top-2) on 8 TRN2 NeuronCores, expert-parallel.

Strategy (sparse dispatch + mixed-precision mm2):
  - Host computes the router (fp32 logits, top-2, softmax renormalize) and
    dispatches: core m owns expert m's weights.
  - Per expert, tokens sort by combine weight. The G=768 highest-weight
    token-expert pairs run fully in bf16. The remainder (low combine
    weight) runs mm1 in bf16 but mm2 in fp8 e4m3 with DoubleRow perf mode
    (2x PE throughput); the fp8 quantization error is suppressed by those
    tokens' small combine weights (overall rel-err ~1.4e-2 < 2e-2).
  - SPMD static shapes per core: G bf16 tokens + 256-token fp8 slot A
    (own expert) + 64-token fp8 slot B (another expert's overflow, that
    expert's weights are DMA'd to this core). Underfilled slots pad with
    zero-weight tokens.
  - Combine weights apply on device (DVE); b2 is added on host (it only
    multiplies combine weights, which the host has).
"""

from contextlib import ExitStack

import ml_dtypes
import numpy as np

P = 128
B, S, H, F, E = 2, 2048, 1024, 4096, 8
T = B * S            # 4096 tokens
KH = H // P          # 8   k-subtiles over H
KF = F // P          # 32  k-subtiles over F

G_DEF = 768          # bf16 tokens per core (multiple of 128)
FA_DEF = 256         # fp8 slot A capacity (own expert, multiple of 64)
FB_DEF = 64          # fp8 slot B capacity (spill expert, multiple of 64)

bf16 = ml_dtypes.bfloat16
f8 = ml_dtypes.float8_e4m3fn
WSCALE = 64.0        # fp8 w2 pre-scale (folded into combine weights)

_CACHE = {}


def _build_nc(G, FA, FB):
    import concourse.mybir as mybir
    import concourse.tile as tile
    from concourse import bacc

    dt = mybir.dt
    AF = mybir.ActivationFunctionType
    DR = mybir.MatmulPerfMode.DoubleRow

    GT = G // P                    # bf16 token tiles (6)
    NT = G + FA + FB               # tokens per core (1088)
    NAT = FA // 64                 # fp8 A mm2 tiles (4)
    NBT = FB // 64                 # fp8 B mm2 tiles (1)

    nc = bacc.Bacc(
        "TRN2", target_bir_lowering=False, debug=False, num_devices=E)

    xb = nc.declare_dram_parameter("xb", [H, NT], dt.bfloat16, isOutput=False)
    w1b = nc.declare_dram_parameter("w1b", [H, F], dt.bfloat16, isOutput=False)
    w2b = nc.declare_dram_parameter("w2b", [F, H], dt.bfloat16, isOutput=False)
    w1s = nc.declare_dram_parameter("w1s", [H, F], dt.bfloat16, isOutput=False)
    w2a = nc.declare_dram_parameter("w2a", [F, H], dt.float8e4, isOutput=False)
    w2s = nc.declare_dram_parameter("w2s", [F, H], dt.float8e4, isOutput=False)
    b1o = nc.declare_dram_parameter("b1o", [P, KF], dt.float32, isOutput=False)
    b1s = nc.declare_dram_parameter("b1s", [P, KF], dt.float32, isOutput=False)
    wmb = nc.declare_dram_parameter("wmb", [P, GT], dt.float32, isOutput=False)
    wma = nc.declare_dram_parameter("wma", [64, NAT], dt.float32, isOutput=False)
    wms = nc.declare_dram_parameter("wms", [64, NBT], dt.float32, isOutput=False)
    yc = nc.declare_dram_parameter("yc", [NT, H], dt.float32, isOutput=True)

    xb_r = xb.rearrange("(k p) t -> p k t", p=P)
    w1b_r = w1b.rearrange("(k p) f -> p k f", p=P)
    w2b_r = w2b.rearrange("(k p) h -> p k h", p=P)
    w1s_r = w1s.rearrange("(k p) f -> p k f", p=P)
    w2a_r = w2a.rearrange("(k p) h -> p k h", p=P)
    w2s_r = w2s.rearrange("(k p) h -> p k h", p=P)

    with ExitStack() as ctx:
        tc = ctx.enter_context(tile.TileContext(nc))
        const = ctx.enter_context(tc.tile_pool(name="const", bufs=1))
        w1pool = ctx.enter_context(tc.tile_pool(name="w1", bufs=8))
        w2pool = ctx.enter_context(tc.tile_pool(name="w2", bufs=2))
        xbp = ctx.enter_context(tc.tile_pool(name="xb", bufs=2))
        h1bp = ctx.enter_context(tc.tile_pool(name="h1b", bufs=1))
        h1fp = ctx.enter_context(tc.tile_pool(name="h1f", bufs=2))
        opool = ctx.enter_context(tc.tile_pool(name="ob", bufs=7))
        p1 = ctx.enter_context(tc.tile_pool(name="p1", bufs=5, space="PSUM"))
        p2 = ctx.enter_context(tc.tile_pool(name="p2", bufs=3, space="PSUM"))

        # ---- x chunk loads ----
        def load_x(t0, csz):
            xt = xbp.tile([P, KH, 512], dt.bfloat16, name="xbt")[:, :, :csz]
            for k in range(KH):
                nc.sync.dma_start(xt[:, k], xb_r[:, k, t0:t0 + csz])
            return xt

        # Startup order: x chunk-0's k=0 plane, then w1b's first 512-f
        # tile, then the rest -- the first matmul is runnable after ~1.1MB.
        xc0 = xbp.tile([P, KH, 512], dt.bfloat16, name="xbt")
        nc.sync.dma_start(xc0[:, 0], xb_r[:, 0, 0:512])

        # w1 weights: 8 x 1MB tiles so matmuls march at 1MB granularity.
        w1b_q = []
        for q in range(8):
            t = w1pool.tile([P, KH, 512], dt.bfloat16, name="w1")
            w1b_q.append(t)
            nc.sync.dma_start(t[:], w1b_r[:, :, q * 512:(q + 1) * 512])
            if q == 0:
                for k in range(1, KH):
                    nc.sync.dma_start(xc0[:, k], xb_r[:, k, 0:512])
                # consts ride after the startup-critical transfers
                b1o_s = const.tile([P, KF], dt.float32)
                nc.sync.dma_start(b1o_s[:], b1o[:])
                b1s_s = const.tile([P, KF], dt.float32)
                nc.sync.dma_start(b1s_s[:], b1s[:])
                wmb_s = const.tile([P, GT], dt.float32)
                nc.sync.dma_start(wmb_s[:], wmb[:])
                wma_s = const.tile([64, NAT], dt.float32)
                nc.sync.dma_start(wma_s[:], wma[:])
                wms_s = const.tile([64, NBT], dt.float32)
                nc.sync.dma_start(wms_s[:], wms[:])
        w2b_t = []
        for half in range(2):
            t = w2pool.tile([P, KF, H // 2], dt.bfloat16, name="w2")
            w2b_t.append(t)
            for hc in range(2):
                nc.sync.dma_start(
                    t[:, :, hc * 256:(hc + 1) * 256],
                    w2b_r[:, :, half * (H // 2) + hc * 256:half * (H // 2) + (hc + 1) * 256])

        # ---- late weights; slot reuse + FIFO order hides their DMA ----
        # w2a reuses w2b half1's slot (dead ~27us before bf16 end), w1s
        # chunks reuse w1b's slots (dead progressively during cA-mm1),
        # w2s reuses w2b half2's slot; issue order matches need order.
        w2a_t = w2pool.tile([P, KF, H], dt.float8e4, name="w2")
        for hc in range(2):
            nc.sync.dma_start(w2a_t[:, :, hc * 512:(hc + 1) * 512],
                              w2a_r[:, :, hc * 512:(hc + 1) * 512])
        w2s_t = w2pool.tile([P, KF, H], dt.float8e4, name="w2")
        for hc in range(2):
            nc.sync.dma_start(w2s_t[:, :, hc * 512:(hc + 1) * 512],
                              w2s_r[:, :, hc * 512:(hc + 1) * 512])
        w1s_q = []
        for q in range(8):
            t = w1pool.tile([P, KH, 512], dt.bfloat16, name="w1")
            w1s_q.append(t)
            nc.sync.dma_start(t[:], w1s_r[:, :, q * 512:(q + 1) * 512])

        # ---- mm1 (bf16) ----
        def mm1(xt, csz, w1t, b1t, h1):
            for ft in range(KF):
                src = w1t[ft // 4]
                ps = p1.tile([P, 512], dt.float32, name="ps1")[:, :csz]
                for k in range(KH):
                    nc.tensor.matmul(
                        ps[:], src[:, k, (ft % 4) * P:(ft % 4 + 1) * P], xt[:, k],
                        start=(k == 0), stop=(k == KH - 1))
                nc.scalar.activation(h1[:, ft], ps[:], AF.Gelu,
                                     bias=b1t[:, ft:ft + 1])

        # ---- mm2 bf16 ----
        def mm2_bf16(h1, t0, csz):
            for ct in range(csz // P):
                gt = t0 // P + ct
                for hh in range(2):
                    src = w2b_t[hh]
                    ps2 = p2.tile([P, 512], dt.float32, name="ps2")
                    for kf in range(KF):
                        nc.tensor.matmul(
                            ps2[:], h1[:, kf, ct * P:(ct + 1) * P], src[:, kf, :],
                            start=(kf == 0), stop=(kf == KF - 1))
                    ob = opool.tile([P, 512], dt.float32, name="ob")
                    nc.vector.tensor_scalar_mul(ob[:], ps2[:], wmb_s[:, gt:gt + 1])
                    nc.sync.dma_start(
                        yc[gt * P:(gt + 1) * P, hh * 512:(hh + 1) * 512], ob[:])

        # ---- mm2 fp8 (DoubleRow) ----
        def mm2_fp8(h1, w2t, wmt, t0, csz, fill=None):
            # fill: optional generator yielding thunks of extra PE work to
            # interleave between psum groups (keeps PE fed when the groups
            # alone would be drained faster than ACT/DVE latency allows).
            for tt in range(csz // 64):
                for hh in range(4):
                    ps2 = p2.tile([P, 512], dt.float32, name="ps2")[:64, :256]
                    for kp in range(KF // 2):
                        nc.tensor.matmul(
                            ps2[:], h1[:, 2 * kp:2 * kp + 2, tt * 64:(tt + 1) * 64],
                            w2t[:, 2 * kp:2 * kp + 2, hh * 256:(hh + 1) * 256],
                            start=(kp == 0), stop=(kp == KF // 2 - 1),
                            perf_mode=DR)
                    ob = opool.tile([P, 512], dt.float32, name="ob")[:64, :256]
                    nc.vector.tensor_scalar_mul(ob[:], ps2[:], wmt[:, tt:tt + 1])
                    nc.sync.dma_start(
                        yc[t0 + tt * 64:t0 + (tt + 1) * 64,
                           hh * 256:(hh + 1) * 256], ob[:])
                    if fill is not None:
                        for thunk in fill(tt * 4 + hh):
                            thunk()

        # ---- schedule ----
        h1 = h1bp.tile([P, KF, 512], dt.bfloat16, name="h1b")
        mm1(xc0, 512, w1b_q, b1o_s, h1)
        mm2_bf16(h1, 0, 512)

        xc1 = load_x(512, 256)
        h1 = h1bp.tile([P, KF, 512], dt.bfloat16, name="h1b")[:, :, :256]
        mm1(xc1, 256, w1b_q, b1o_s, h1)
        mm2_bf16(h1, 512, 256)

        xca = load_x(G, FA)
        h1a = h1fp.tile([P, KF, FA], dt.float8e4, name="h1f")
        mm1(xca, FA, w1b_q, b1o_s, h1a)

        # cB's tiny mm1 is ACT-latency-paced on its own (22ns PE bubbles
        # that also reset the p-state ramp); interleave its f-tiles
        # between cA-mm2 psum groups so PE stays saturated.
        xcb = load_x(G + FA, FB)
        h1b2 = h1fp.tile([P, KF, FA], dt.float8e4, name="h1f")[:, :, :FB]

        def emit_cb_ft(ft):
            src = w1s_q[ft // 4]
            ps = p1.tile([P, 512], dt.float32, name="ps1")[:, :FB]
            for k in range(KH):
                nc.tensor.matmul(
                    ps[:], src[:, k, (ft % 4) * P:(ft % 4 + 1) * P], xcb[:, k],
                    start=(k == 0), stop=(k == KH - 1))
            nc.scalar.activation(h1b2[:, ft], ps[:], AF.Gelu,
                                 bias=b1s_s[:, ft:ft + 1])

        def fill_cb(group_idx):
            ft0 = group_idx * 2
            return [(lambda ft=ft: emit_cb_ft(ft))
                    for ft in range(ft0, min(ft0 + 2, KF))]

        mm2_fp8(h1a, w2a_t, wma_s, G, FA, fill=fill_cb)
        mm2_fp8(h1b2, w2s_t, wms_s, G + FA, FB)
    return nc


def _get_nc(G, FA, FB):
    key = (G, FA, FB)
    if key not in _CACHE:
        nc = _build_nc(G, FA, FB)
        nc.finalize()
        _CACHE[key] = nc
    return _CACHE[key]


def dispatch(hidden_states, router_w, router_b):
    """Host router: top-2 ids + renormalized combine weights per token."""
    x = np.asarray(hidden_states, dtype=np.float32).reshape(T, H)
    logits = x @ np.asarray(router_w, dtype=np.float32)
    logits = logits + np.asarray(router_b, dtype=np.float32)
    part = np.argpartition(logits, E - 2, axis=1)[:, E - 2:]      # [T,2] unordered
    pv = np.take_along_axis(logits, part, axis=1)
    swap = pv[:, 0] > pv[:, 1]
    i1 = np.where(swap, part[:, 0], part[:, 1])
    i2 = np.where(swap, part[:, 1], part[:, 0])
    l1 = logits[np.arange(T), i1]
    l2 = logits[np.arange(T), i2]
    e2 = np.exp((l2 - l1).astype(np.float64))
    wt1 = (1.0 / (1.0 + e2)).astype(np.float32)
    wt2 = (e2 / (1.0 + e2)).astype(np.float32)
    return x, i1, i2, wt1, wt2


def plan(i1, i2, wt1, wt2, G, FA, FB):
    """Token->(core, group) assignment."""
    bf_tok, bf_wt, a_tok, a_wt, spill = [], [], [], [], []
    for m in range(E):
        tk = np.concatenate([np.where(i1 == m)[0], np.where(i2 == m)[0]])
        wt = np.concatenate([wt1[i1 == m], wt2[i2 == m]])
        o = np.argsort(-wt)
        tk, wt = tk[o], wt[o]
        bf_tok.append(tk[:G])
        bf_wt.append(wt[:G])
        a_tok.append(tk[G:G + FA])
        a_wt.append(wt[G:G + FA])
        rest_t, rest_w = tk[G + FA:], wt[G + FA:]
        for s0 in range(0, len(rest_t), FB):
            spill.append((m, rest_t[s0:s0 + FB], rest_w[s0:s0 + FB]))
    assert len(spill) <= E, f"spill slots {len(spill)} > {E}"
    while len(spill) < E:
        spill.append((0, np.zeros(0, np.int64), np.zeros(0, np.float32)))
    return bf_tok, bf_wt, a_tok, a_wt, spill


def make_in_maps(hidden_states, router_w, router_b, w1, b1, w2, b2,
                 G=G_DEF, FA=FA_DEF, FB=FB_DEF):
    x, i1, i2, wt1, wt2 = dispatch(hidden_states, router_w, router_b)
    bf_tok, bf_wt, a_tok, a_wt, spill = plan(i1, i2, wt1, wt2, G, FA, FB)
    w1 = np.asarray(w1, dtype=np.float32)
    w2 = np.asarray(w2, dtype=np.float32)
    b1 = np.asarray(b1, dtype=np.float32)
    b2 = np.asarray(b2, dtype=np.float32)
    xt = np.ascontiguousarray(x.T)                      # [H, T]
    GT = G // P

    def wcol(wts, cap, rows, scale):
        ncol = cap // rows
        out = np.zeros((rows, ncol), dtype=np.float32)
        wv = np.zeros(cap, dtype=np.float32)
        wv[:len(wts)] = wts * scale
        for c in range(ncol):
            out[:, c] = wv[c * rows:(c + 1) * rows]
        return out

    w1b16 = [np.ascontiguousarray(w1[m].astype(bf16)) for m in range(E)]
    w2f8 = [np.ascontiguousarray((w2[m] * WSCALE).astype(f8)) for m in range(E)]
    b1r = [np.ascontiguousarray(b1[m].reshape(KF, P).T) for m in range(E)]

    in_maps = []
    for m in range(E):
        sm, st, sw = spill[m]
        xbm = np.zeros((H, G + FA + FB), dtype=bf16)
        xbm[:, :len(bf_tok[m])] = xt[:, bf_tok[m]].astype(bf16)
        xbm[:, G:G + len(a_tok[m])] = xt[:, a_tok[m]].astype(bf16)
        xbm[:, G + FA:G + FA + len(st)] = xt[:, st].astype(bf16)
        in_maps.append({
            "xb": xbm,
            "w1b": w1b16[m],
            "w2b": np.ascontiguousarray(w2[m].astype(bf16)),
            "w1s": w1b16[sm],
            "w2a": w2f8[m], "w2s": w2f8[sm],
            "b1o": b1r[m], "b1s": b1r[sm],
            "wmb": wcol(bf_wt[m], GT * P, P, 1.0),
            "wma": wcol(a_wt[m], FA, 64, 1.0 / WSCALE),
            "wms": wcol(sw, FB, 64, 1.0 / WSCALE),
        })
    meta = (bf_tok, a_tok, spill, i1, i2, wt1, wt2)
    return in_maps, meta


def run_device(in_maps, G=G_DEF, FA=FA_DEF, FB=FB_DEF):
    from concourse.bass_utils import run_bass_kernel_spmd

    nc = _get_nc(G, FA, FB)
    res = run_bass_kernel_spmd(nc, in_maps, core_ids=list(range(E)))
    return res.results


def kernel(hidden_states, router_w, router_b, w1, b1, w2, b2):
    G, FA, FB = G_DEF, FA_DEF, FB_DEF
    in_maps, meta = make_in_maps(
        hidden_states, router_w, router_b, w1, b1, w2, b2, G, FA, FB)
    bf_tok, a_tok, spill, i1, i2, wt1, wt2 = meta
    b2 = np.asarray(b2, dtype=np.float32)
    # One retry guards against rare transient NRT/axon failures.
    last_err = None
    for attempt in range(3):
        try:
            results = run_device(in_maps, G, FA, FB)
        except Exception as e:
            last_err = e
            import time as _time
            _time.sleep(10)
            continue
        acc = np.zeros((T, H), dtype=np.float32)
        for m in range(E):
            ycm = np.asarray(results[m]["yc"], dtype=np.float32)
            acc[bf_tok[m]] += ycm[:len(bf_tok[m])]
            if len(a_tok[m]):
                acc[a_tok[m]] += ycm[G:G + len(a_tok[m])]
            sm, st, sw = spill[m]
            if len(st):
                acc[st] += ycm[G + FA:G + FA + len(st)]
        # b2 contribution (combine-weighted), host-side
        acc += wt1[:, None] * b2[i1] + wt2[:, None] * b2[i2]
        if np.isfinite(acc).all() and np.abs(acc).max() < 1e4:
            return acc.reshape(B, S, H)
    if last_err is not None:
        raise last_err
    return acc.reshape(B, S, H)


# revision 20
# speedup vs baseline: 1.1480x; 1.0100x over previous
"""MoE layer (8 experts, top-2) on 8 TRN2 NeuronCores, expert-parallel.

Strategy (sparse dispatch + mixed-precision mm2):
  - Host computes the router (fp32 logits, top-2, softmax renormalize) and
    dispatches: core m owns expert m's weights.
  - Per expert, tokens sort by combine weight. The G=768 highest-weight
    token-expert pairs run fully in bf16. The remainder (low combine
    weight) runs mm1 in bf16 but mm2 in fp8 e4m3 with DoubleRow perf mode
    (2x PE throughput); the fp8 quantization error is suppressed by those
    tokens' small combine weights (overall rel-err ~1.4e-2 < 2e-2).
  - SPMD static shapes per core: G bf16 tokens + 256-token fp8 slot A
    (own expert) + 64-token fp8 slot B (another expert's overflow, that
    expert's weights are DMA'd to this core). Underfilled slots pad with
    zero-weight tokens.
  - Combine weights apply on device (DVE); b2 is added on host (it only
    multiplies combine weights, which the host has).
"""

from contextlib import ExitStack

import ml_dtypes
import numpy as np

P = 128
B, S, H, F, E = 2, 2048, 1024, 4096, 8
T = B * S            # 4096 tokens
KH = H // P          # 8   k-subtiles over H
KF = F // P          # 32  k-subtiles over F

G_DEF = 768          # bf16 tokens per core (multiple of 128)
FA_DEF = 256         # fp8 slot A capacity (own expert, multiple of 64)
FB_DEF = 64          # fp8 slot B capacity (spill expert, multiple of 64)

bf16 = ml_dtypes.bfloat16
f8 = ml_dtypes.float8_e4m3fn
WSCALE = 64.0        # fp8 w2 pre-scale (folded into combine weights)

_CACHE = {}


def _build_nc(G, FA, FB):
    import concourse.mybir as mybir
    import concourse.tile as tile
    from concourse import bacc

    dt = mybir.dt
    AF = mybir.ActivationFunctionType
    DR = mybir.MatmulPerfMode.DoubleRow

    GT = G // P                    # bf16 token tiles (6)
    NT = G + FA + FB               # tokens per core (1088)
    NAT = FA // 64                 # fp8 A mm2 tiles (4)
    NBT = FB // 64                 # fp8 B mm2 tiles (1)

    nc = bacc.Bacc(
        "TRN2", target_bir_lowering=False, debug=False, num_devices=E)

    xb = nc.declare_dram_parameter("xb", [H, NT], dt.bfloat16, isOutput=False)
    w1b = nc.declare_dram_parameter("w1b", [H, F], dt.bfloat16, isOutput=False)
    w2b = nc.declare_dram_parameter("w2b", [F, H], dt.bfloat16, isOutput=False)
    w1s = nc.declare_dram_parameter("w1s", [H, F], dt.bfloat16, isOutput=False)
    w2a = nc.declare_dram_parameter("w2a", [F, H], dt.float8e4, isOutput=False)
    w2s = nc.declare_dram_parameter("w2s", [F, H], dt.float8e4, isOutput=False)
    b1o = nc.declare_dram_parameter("b1o", [P, KF], dt.float32, isOutput=False)
    b1s = nc.declare_dram_parameter("b1s", [P, KF], dt.float32, isOutput=False)
    wmb = nc.declare_dram_parameter("wmb", [P, GT], dt.float32, isOutput=False)
    wma = nc.declare_dram_parameter("wma", [64, NAT], dt.float32, isOutput=False)
    wms = nc.declare_dram_parameter("wms", [64, NBT], dt.float32, isOutput=False)
    yc = nc.declare_dram_parameter("yc", [NT, H], dt.float32, isOutput=True)

    xb_r = xb.rearrange("(k p) t -> p k t", p=P)
    w1b_r = w1b.rearrange("(k p) f -> p k f", p=P)
    w2b_r = w2b.rearrange("(k p) h -> p k h", p=P)
    w1s_r = w1s.rearrange("(k p) f -> p k f", p=P)
    w2a_r = w2a.rearrange("(k p) h -> p k h", p=P)
    w2s_r = w2s.rearrange("(k p) h -> p k h", p=P)

    with ExitStack() as ctx:
        tc = ctx.enter_context(tile.TileContext(nc))
        const = ctx.enter_context(tc.tile_pool(name="const", bufs=1))
        w1pool = ctx.enter_context(tc.tile_pool(name="w1", bufs=8))
        w2pool = ctx.enter_context(tc.tile_pool(name="w2", bufs=2))
        xbp = ctx.enter_context(tc.tile_pool(name="xb", bufs=2))
        h1bp = ctx.enter_context(tc.tile_pool(name="h1b", bufs=1))
        h1fp = ctx.enter_context(tc.tile_pool(name="h1f", bufs=2))
        opool = ctx.enter_context(tc.tile_pool(name="ob", bufs=7))
        p1 = ctx.enter_context(tc.tile_pool(name="p1", bufs=5, space="PSUM"))
        p2 = ctx.enter_context(tc.tile_pool(name="p2", bufs=3, space="PSUM"))

        # ---- x chunk loads ----
        def load_x(t0, csz):
            xt = xbp.tile([P, KH, 512], dt.bfloat16, name="xbt")[:, :, :csz]
            for k in range(KH):
                nc.sync.dma_start(xt[:, k], xb_r[:, k, t0:t0 + csz])
            return xt

        # Startup order: x chunk-0's k=0 plane, then w1b's first 512-f
        # tile, then the rest -- the first matmul is runnable after ~1.1MB.
        xc0 = xbp.tile([P, KH, 512], dt.bfloat16, name="xbt")
        nc.sync.dma_start(xc0[:, 0], xb_r[:, 0, 0:512])

        # w1 weights: 8 x 1MB tiles so matmuls march at 1MB granularity.
        w1b_q = []
        for q in range(8):
            t = w1pool.tile([P, KH, 512], dt.bfloat16, name="w1")
            w1b_q.append(t)
            nc.sync.dma_start(t[:], w1b_r[:, :, q * 512:(q + 1) * 512])
            if q == 0:
                for k in range(1, KH):
                    nc.sync.dma_start(xc0[:, k], xb_r[:, k, 0:512])
                # consts ride after the startup-critical transfers
                b1o_s = const.tile([P, KF], dt.float32)
                nc.sync.dma_start(b1o_s[:], b1o[:])
                b1s_s = const.tile([P, KF], dt.float32)
                nc.sync.dma_start(b1s_s[:], b1s[:])
                wmb_s = const.tile([P, GT], dt.float32)
                nc.sync.dma_start(wmb_s[:], wmb[:])
                wma_s = const.tile([64, NAT], dt.float32)
                nc.sync.dma_start(wma_s[:], wma[:])
                wms_s = const.tile([64, NBT], dt.float32)
                nc.sync.dma_start(wms_s[:], wms[:])
        w2b_t = []
        for half in range(2):
            t = w2pool.tile([P, KF, H // 2], dt.bfloat16, name="w2")
            w2b_t.append(t)
            for hc in range(2):
                nc.sync.dma_start(
                    t[:, :, hc * 256:(hc + 1) * 256],
                    w2b_r[:, :, half * (H // 2) + hc * 256:half * (H // 2) + (hc + 1) * 256])

        # Remaining x chunks: issued ahead of the slot-blocked weight DMAs
        # so they don't queue behind them. Pool rotation: xc1 -> slot 1,
        # xca -> slot 0 (waits c0-mm1), xcb -> slot 1 (waits c1-mm1).
        xc1 = load_x(512, G - 512)
        xca = load_x(G, FA)
        xcb = load_x(G + FA, FB)

        # ---- late weights; slot reuse + FIFO order hides their DMA ----
        # w2a reuses w2b half1's slot (dead ~27us before bf16 end), w1s
        # chunks reuse w1b's slots (dead progressively during cA-mm1),
        # w2s reuses w2b half2's slot; issue order matches need order.
        w2a_t = w2pool.tile([P, KF, H], dt.float8e4, name="w2")
        for hc in range(2):
            nc.sync.dma_start(w2a_t[:, :, hc * 512:(hc + 1) * 512],
                              w2a_r[:, :, hc * 512:(hc + 1) * 512])
        w2s_t = w2pool.tile([P, KF, H], dt.float8e4, name="w2")
        for hc in range(2):
            nc.sync.dma_start(w2s_t[:, :, hc * 512:(hc + 1) * 512],
                              w2s_r[:, :, hc * 512:(hc + 1) * 512])
        w1s_q = []
        for q in range(8):
            t = w1pool.tile([P, KH, 512], dt.bfloat16, name="w1")
            w1s_q.append(t)
            nc.sync.dma_start(t[:], w1s_r[:, :, q * 512:(q + 1) * 512])

        # ---- mm1 (bf16) ----
        def mm1(xt, csz, w1t, b1t, h1):
            for ft in range(KF):
                src = w1t[ft // 4]
                ps = p1.tile([P, 512], dt.float32, name="ps1")[:, :csz]
                for k in range(KH):
                    nc.tensor.matmul(
                        ps[:], src[:, k, (ft % 4) * P:(ft % 4 + 1) * P], xt[:, k],
                        start=(k == 0), stop=(k == KH - 1))
                nc.scalar.activation(h1[:, ft], ps[:], AF.Gelu,
                                     bias=b1t[:, ft:ft + 1])

        # ---- mm2 bf16 ----
        def mm2_bf16(h1, t0, csz):
            # hh-major so w2b half-tiles die as early as possible (their
            # slots host the fp8 w2 loads).
            for hh in range(2):
                src = w2b_t[hh]
                for ct in range(csz // P):
                    gt = t0 // P + ct
                    ps2 = p2.tile([P, 512], dt.float32, name="ps2")
                    for kf in range(KF):
                        nc.tensor.matmul(
                            ps2[:], h1[:, kf, ct * P:(ct + 1) * P], src[:, kf, :],
                            start=(kf == 0), stop=(kf == KF - 1))
                    ob = opool.tile([P, 512], dt.float32, name="ob")
                    nc.vector.tensor_scalar_mul(ob[:], ps2[:], wmb_s[:, gt:gt + 1])
                    nc.sync.dma_start(
                        yc[gt * P:(gt + 1) * P, hh * 512:(hh + 1) * 512], ob[:])

        # ---- mm2 fp8 (DoubleRow) ----
        def mm2_fp8(h1, w2t, wmt, t0, csz, fill=None):
            # fill: optional generator yielding thunks of extra PE work to
            # interleave between psum groups (keeps PE fed when the groups
            # alone would be drained faster than ACT/DVE latency allows).
            for tt in range(csz // 64):
                for hh in range(4):
                    ps2 = p2.tile([P, 512], dt.float32, name="ps2")[:64, :256]
                    for kp in range(KF // 2):
                        nc.tensor.matmul(
                            ps2[:], h1[:, 2 * kp:2 * kp + 2, tt * 64:(tt + 1) * 64],
                            w2t[:, 2 * kp:2 * kp + 2, hh * 256:(hh + 1) * 256],
                            start=(kp == 0), stop=(kp == KF // 2 - 1),
                            perf_mode=DR)
                    ob = opool.tile([P, 512], dt.float32, name="ob")[:64, :256]
                    nc.vector.tensor_scalar_mul(ob[:], ps2[:], wmt[:, tt:tt + 1])
                    nc.sync.dma_start(
                        yc[t0 + tt * 64:t0 + (tt + 1) * 64,
                           hh * 256:(hh + 1) * 256], ob[:])
                    if fill is not None:
                        for thunk in fill(tt * 4 + hh):
                            thunk()

        # ---- schedule ----
        h1 = h1bp.tile([P, KF, 512], dt.bfloat16, name="h1b")
        mm1(xc0, 512, w1b_q, b1o_s, h1)
        mm2_bf16(h1, 0, 512)

        h1 = h1bp.tile([P, KF, 512], dt.bfloat16, name="h1b")[:, :, :256]
        mm1(xc1, 256, w1b_q, b1o_s, h1)
        mm2_bf16(h1, 512, 256)

        h1a = h1fp.tile([P, KF, FA], dt.float8e4, name="h1f")
        mm1(xca, FA, w1b_q, b1o_s, h1a)

        # cB's tiny mm1 is ACT-latency-paced on its own (22ns PE bubbles
        # that also reset the p-state ramp); interleave its f-tiles
        # between cA-mm2 psum groups so PE stays saturated.
        h1b2 = h1fp.tile([P, KF, FA], dt.float8e4, name="h1f")[:, :, :FB]

        def emit_cb_ft(ft):
            src = w1s_q[ft // 4]
            ps = p1.tile([P, 512], dt.float32, name="ps1")[:, :FB]
            for k in range(KH):
                nc.tensor.matmul(
                    ps[:], src[:, k, (ft % 4) * P:(ft % 4 + 1) * P], xcb[:, k],
                    start=(k == 0), stop=(k == KH - 1))
            nc.scalar.activation(h1b2[:, ft], ps[:], AF.Gelu,
                                 bias=b1s_s[:, ft:ft + 1])

        def fill_cb(group_idx):
            ft0 = group_idx * 2
            return [(lambda ft=ft: emit_cb_ft(ft))
                    for ft in range(ft0, min(ft0 + 2, KF))]

        mm2_fp8(h1a, w2a_t, wma_s, G, FA, fill=fill_cb)
        mm2_fp8(h1b2, w2s_t, wms_s, G + FA, FB)
    return nc


def _get_nc(G, FA, FB):
    key = (G, FA, FB)
    if key not in _CACHE:
        nc = _build_nc(G, FA, FB)
        nc.finalize()
        _CACHE[key] = nc
    return _CACHE[key]


def dispatch(hidden_states, router_w, router_b):
    """Host router: top-2 ids + renormalized combine weights per token."""
    x = np.asarray(hidden_states, dtype=np.float32).reshape(T, H)
    logits = x @ np.asarray(router_w, dtype=np.float32)
    logits = logits + np.asarray(router_b, dtype=np.float32)
    part = np.argpartition(logits, E - 2, axis=1)[:, E - 2:]      # [T,2] unordered
    pv = np.take_along_axis(logits, part, axis=1)
    swap = pv[:, 0] > pv[:, 1]
    i1 = np.where(swap, part[:, 0], part[:, 1])
    i2 = np.where(swap, part[:, 1], part[:, 0])
    l1 = logits[np.arange(T), i1]
    l2 = logits[np.arange(T), i2]
    e2 = np.exp((l2 - l1).astype(np.float64))
    wt1 = (1.0 / (1.0 + e2)).astype(np.float32)
    wt2 = (e2 / (1.0 + e2)).astype(np.float32)
    return x, i1, i2, wt1, wt2


def plan(i1, i2, wt1, wt2, G, FA, FB):
    """Token->(core, group) assignment."""
    bf_tok, bf_wt, a_tok, a_wt, spill = [], [], [], [], []
    for m in range(E):
        tk = np.concatenate([np.where(i1 == m)[0], np.where(i2 == m)[0]])
        wt = np.concatenate([wt1[i1 == m], wt2[i2 == m]])
        o = np.argsort(-wt)
        tk, wt = tk[o], wt[o]
        bf_tok.append(tk[:G])
        bf_wt.append(wt[:G])
        a_tok.append(tk[G:G + FA])
        a_wt.append(wt[G:G + FA])
        rest_t, rest_w = tk[G + FA:], wt[G + FA:]
        for s0 in range(0, len(rest_t), FB):
            spill.append((m, rest_t[s0:s0 + FB], rest_w[s0:s0 + FB]))
    assert len(spill) <= E, f"spill slots {len(spill)} > {E}"
    while len(spill) < E:
        spill.append((0, np.zeros(0, np.int64), np.zeros(0, np.float32)))
    return bf_tok, bf_wt, a_tok, a_wt, spill


def make_in_maps(hidden_states, router_w, router_b, w1, b1, w2, b2,
                 G=G_DEF, FA=FA_DEF, FB=FB_DEF):
    x, i1, i2, wt1, wt2 = dispatch(hidden_states, router_w, router_b)
    bf_tok, bf_wt, a_tok, a_wt, spill = plan(i1, i2, wt1, wt2, G, FA, FB)
    w1 = np.asarray(w1, dtype=np.float32)
    w2 = np.asarray(w2, dtype=np.float32)
    b1 = np.asarray(b1, dtype=np.float32)
    b2 = np.asarray(b2, dtype=np.float32)
    xt = np.ascontiguousarray(x.T)                      # [H, T]
    GT = G // P

    def wcol(wts, cap, rows, scale):
        ncol = cap // rows
        out = np.zeros((rows, ncol), dtype=np.float32)
        wv = np.zeros(cap, dtype=np.float32)
        wv[:len(wts)] = wts * scale
        for c in range(ncol):
            out[:, c] = wv[c * rows:(c + 1) * rows]
        return out

    w1b16 = [np.ascontiguousarray(w1[m].astype(bf16)) for m in range(E)]
    w2f8 = [np.ascontiguousarray((w2[m] * WSCALE).astype(f8)) for m in range(E)]
    b1r = [np.ascontiguousarray(b1[m].reshape(KF, P).T) for m in range(E)]

    in_maps = []
    for m in range(E):
        sm, st, sw = spill[m]
        xbm = np.zeros((H, G + FA + FB), dtype=bf16)
        xbm[:, :len(bf_tok[m])] = xt[:, bf_tok[m]].astype(bf16)
        xbm[:, G:G + len(a_tok[m])] = xt[:, a_tok[m]].astype(bf16)
        xbm[:, G + FA:G + FA + len(st)] = xt[:, st].astype(bf16)
        in_maps.append({
            "xb": xbm,
            "w1b": w1b16[m],
            "w2b": np.ascontiguousarray(w2[m].astype(bf16)),
            "w1s": w1b16[sm],
            "w2a": w2f8[m], "w2s": w2f8[sm],
            "b1o": b1r[m], "b1s": b1r[sm],
            "wmb": wcol(bf_wt[m], GT * P, P, 1.0),
            "wma": wcol(a_wt[m], FA, 64, 1.0 / WSCALE),
            "wms": wcol(sw, FB, 64, 1.0 / WSCALE),
        })
    meta = (bf_tok, a_tok, spill, i1, i2, wt1, wt2)
    return in_maps, meta


def run_device(in_maps, G=G_DEF, FA=FA_DEF, FB=FB_DEF):
    from concourse.bass_utils import run_bass_kernel_spmd

    nc = _get_nc(G, FA, FB)
    res = run_bass_kernel_spmd(nc, in_maps, core_ids=list(range(E)))
    return res.results


def kernel(hidden_states, router_w, router_b, w1, b1, w2, b2):
    G, FA, FB = G_DEF, FA_DEF, FB_DEF
    in_maps, meta = make_in_maps(
        hidden_states, router_w, router_b, w1, b1, w2, b2, G, FA, FB)
    bf_tok, a_tok, spill, i1, i2, wt1, wt2 = meta
    b2 = np.asarray(b2, dtype=np.float32)
    # One retry guards against rare transient NRT/axon failures.
    last_err = None
    for attempt in range(3):
        try:
            results = run_device(in_maps, G, FA, FB)
        except Exception as e:
            last_err = e
            import time as _time
            _time.sleep(10)
            continue
        acc = np.zeros((T, H), dtype=np.float32)
        for m in range(E):
            ycm = np.asarray(results[m]["yc"], dtype=np.float32)
            acc[bf_tok[m]] += ycm[:len(bf_tok[m])]
            if len(a_tok[m]):
                acc[a_tok[m]] += ycm[G:G + len(a_tok[m])]
            sm, st, sw = spill[m]
            if len(st):
                acc[st] += ycm[G + FA:G + FA + len(st)]
        # b2 contribution (combine-weighted), host-side
        acc += wt1[:, None] * b2[i1] + wt2[:, None] * b2[i2]
        if np.isfinite(acc).all() and np.abs(acc).max() < 1e4:
            return acc.reshape(B, S, H)
    if last_err is not None:
        raise last_err
    return acc.reshape(B, S, H)


# revision 21
# speedup vs baseline: 1.1643x; 1.0142x over previous
"""MoE layer (8 experts, top-2) on 8 TRN2 NeuronCores, expert-parallel.

Strategy (sparse dispatch + mixed-precision mm2):
  - Host computes the router (fp32 logits, top-2, softmax renormalize) and
    dispatches: core m owns expert m's weights.
  - Per expert, tokens sort by combine weight. The G=768 highest-weight
    token-expert pairs run fully in bf16. The remainder (low combine
    weight) runs mm1 in bf16 but mm2 in fp8 e4m3 with DoubleRow perf mode
    (2x PE throughput); the fp8 quantization error is suppressed by those
    tokens' small combine weights (overall rel-err ~1.4e-2 < 2e-2).
  - SPMD static shapes per core: G bf16 tokens + 256-token fp8 slot A
    (own expert) + 64-token fp8 slot B (another expert's overflow, that
    expert's weights are DMA'd to this core). Underfilled slots pad with
    zero-weight tokens.
  - Combine weights apply on device (DVE); b2 is added on host (it only
    multiplies combine weights, which the host has).
"""

from contextlib import ExitStack

import ml_dtypes
import numpy as np

P = 128
B, S, H, F, E = 2, 2048, 1024, 4096, 8
T = B * S            # 4096 tokens
KH = H // P          # 8   k-subtiles over H
KF = F // P          # 32  k-subtiles over F

G_DEF = 768          # bf16 tokens per core (multiple of 128)
FA_DEF = 256         # fp8 slot A capacity (own expert, multiple of 64)
FB_DEF = 64          # fp8 slot B capacity (spill expert, multiple of 64)

bf16 = ml_dtypes.bfloat16
f8 = ml_dtypes.float8_e4m3fn
WSCALE = 64.0        # fp8 w2 pre-scale (folded into combine weights)

_CACHE = {}


def _build_nc(G, FA, FB):
    import concourse.mybir as mybir
    import concourse.tile as tile
    from concourse import bacc

    dt = mybir.dt
    AF = mybir.ActivationFunctionType
    DR = mybir.MatmulPerfMode.DoubleRow

    GT = G // P                    # bf16 token tiles (6)
    NT = G + FA + FB               # tokens per core (1088)
    NAT = FA // 64                 # fp8 A mm2 tiles (4)
    NBT = FB // 64                 # fp8 B mm2 tiles (1)

    nc = bacc.Bacc(
        "TRN2", target_bir_lowering=False, debug=False, num_devices=E)

    xb = nc.declare_dram_parameter("xb", [H, NT], dt.bfloat16, isOutput=False)
    w1b = nc.declare_dram_parameter("w1b", [H, F], dt.bfloat16, isOutput=False)
    w2b = nc.declare_dram_parameter("w2b", [F, H], dt.bfloat16, isOutput=False)
    w1s = nc.declare_dram_parameter("w1s", [H, F], dt.bfloat16, isOutput=False)
    w2a = nc.declare_dram_parameter("w2a", [F, H], dt.float8e4, isOutput=False)
    w2s = nc.declare_dram_parameter("w2s", [F, H], dt.float8e4, isOutput=False)
    b1o = nc.declare_dram_parameter("b1o", [P, KF], dt.float32, isOutput=False)
    b1s = nc.declare_dram_parameter("b1s", [P, KF], dt.float32, isOutput=False)
    wmb = nc.declare_dram_parameter("wmb", [P, GT], dt.float32, isOutput=False)
    wma = nc.declare_dram_parameter("wma", [64, NAT], dt.float32, isOutput=False)
    wms = nc.declare_dram_parameter("wms", [64, NBT], dt.float32, isOutput=False)
    yc = nc.declare_dram_parameter("yc", [NT, H], dt.float32, isOutput=True)

    xb_r = xb.rearrange("(k p) t -> p k t", p=P)
    w1b_r = w1b.rearrange("(k p) f -> p k f", p=P)
    w2b_r = w2b.rearrange("(k p) h -> p k h", p=P)
    w1s_r = w1s.rearrange("(k p) f -> p k f", p=P)
    w2a_r = w2a.rearrange("(k p) h -> p k h", p=P)
    w2s_r = w2s.rearrange("(k p) h -> p k h", p=P)

    with ExitStack() as ctx:
        tc = ctx.enter_context(tile.TileContext(nc))
        const = ctx.enter_context(tc.tile_pool(name="const", bufs=1))
        w1pool = ctx.enter_context(tc.tile_pool(name="w1", bufs=8))
        w2pool = ctx.enter_context(tc.tile_pool(name="w2", bufs=2))
        xbp = ctx.enter_context(tc.tile_pool(name="xb", bufs=2))
        h1bp = ctx.enter_context(tc.tile_pool(name="h1b", bufs=1))
        h1fp = ctx.enter_context(tc.tile_pool(name="h1f", bufs=2))
        opool = ctx.enter_context(tc.tile_pool(name="ob", bufs=7))
        p1 = ctx.enter_context(tc.tile_pool(name="p1", bufs=5, space="PSUM"))
        p2 = ctx.enter_context(tc.tile_pool(name="p2", bufs=3, space="PSUM"))

        # ---- PE warmup ----
        # The cost model ramps PE 0.65->1.2->2.4 GHz over the first 3us of
        # continuous execution. Fill the initial DMA wait (~6us) with dummy
        # matmuls on memset data so real matmuls start at full clock.
        wrm = const.tile([P, 640], dt.bfloat16)
        nc.vector.memset(wrm[:], 0.0)
        for i in range(24):
            pw = p1.tile([P, 512], dt.float32, name="ps1")
            nc.tensor.matmul(pw[:], wrm[:, :P], wrm[:, P:P + 512],
                             start=True, stop=True)

        # ---- x chunk loads ----
        def load_x(t0, csz):
            xt = xbp.tile([P, KH, 512], dt.bfloat16, name="xbt")[:, :, :csz]
            for k in range(KH):
                nc.sync.dma_start(xt[:, k], xb_r[:, k, t0:t0 + csz])
            return xt

        # Startup order: x chunk-0's k=0 plane, then w1b's first 512-f
        # tile, then the rest -- the first matmul is runnable after ~1.1MB.
        xc0 = xbp.tile([P, KH, 512], dt.bfloat16, name="xbt")
        nc.sync.dma_start(xc0[:, 0], xb_r[:, 0, 0:512])

        # w1 weights: 8 x 1MB tiles so matmuls march at 1MB granularity.
        w1b_q = []
        for q in range(8):
            t = w1pool.tile([P, KH, 512], dt.bfloat16, name="w1")
            w1b_q.append(t)
            nc.sync.dma_start(t[:], w1b_r[:, :, q * 512:(q + 1) * 512])
            if q == 0:
                for k in range(1, KH):
                    nc.sync.dma_start(xc0[:, k], xb_r[:, k, 0:512])
                # consts ride after the startup-critical transfers
                b1o_s = const.tile([P, KF], dt.float32)
                nc.sync.dma_start(b1o_s[:], b1o[:])
                b1s_s = const.tile([P, KF], dt.float32)
                nc.sync.dma_start(b1s_s[:], b1s[:])
                wmb_s = const.tile([P, GT], dt.float32)
                nc.sync.dma_start(wmb_s[:], wmb[:])
                wma_s = const.tile([64, NAT], dt.float32)
                nc.sync.dma_start(wma_s[:], wma[:])
                wms_s = const.tile([64, NBT], dt.float32)
                nc.sync.dma_start(wms_s[:], wms[:])
        w2b_t = []
        for half in range(2):
            t = w2pool.tile([P, KF, H // 2], dt.bfloat16, name="w2")
            w2b_t.append(t)
            for hc in range(2):
                nc.sync.dma_start(
                    t[:, :, hc * 256:(hc + 1) * 256],
                    w2b_r[:, :, half * (H // 2) + hc * 256:half * (H // 2) + (hc + 1) * 256])

        # Remaining x chunks: issued ahead of the slot-blocked weight DMAs
        # so they don't queue behind them. Pool rotation: xc1 -> slot 1,
        # xca -> slot 0 (waits c0-mm1), xcb -> slot 1 (waits c1-mm1).
        xc1 = load_x(512, G - 512)
        xca = load_x(G, FA)
        xcb = load_x(G + FA, FB)

        # ---- late weights; slot reuse + FIFO order hides their DMA ----
        # w2a reuses w2b half1's slot (dead ~27us before bf16 end), w1s
        # chunks reuse w1b's slots (dead progressively during cA-mm1),
        # w2s reuses w2b half2's slot; issue order matches need order.
        w2a_t = w2pool.tile([P, KF, H], dt.float8e4, name="w2")
        for hc in range(2):
            nc.sync.dma_start(w2a_t[:, :, hc * 512:(hc + 1) * 512],
                              w2a_r[:, :, hc * 512:(hc + 1) * 512])
        w2s_t = w2pool.tile([P, KF, H], dt.float8e4, name="w2")
        for hc in range(2):
            nc.sync.dma_start(w2s_t[:, :, hc * 512:(hc + 1) * 512],
                              w2s_r[:, :, hc * 512:(hc + 1) * 512])
        w1s_q = []
        for q in range(8):
            t = w1pool.tile([P, KH, 512], dt.bfloat16, name="w1")
            w1s_q.append(t)
            nc.sync.dma_start(t[:], w1s_r[:, :, q * 512:(q + 1) * 512])

        # ---- mm1 (bf16) ----
        def mm1(xt, csz, w1t, b1t, h1):
            for ft in range(KF):
                src = w1t[ft // 4]
                ps = p1.tile([P, 512], dt.float32, name="ps1")[:, :csz]
                for k in range(KH):
                    nc.tensor.matmul(
                        ps[:], src[:, k, (ft % 4) * P:(ft % 4 + 1) * P], xt[:, k],
                        start=(k == 0), stop=(k == KH - 1))
                nc.scalar.activation(h1[:, ft], ps[:], AF.Gelu,
                                     bias=b1t[:, ft:ft + 1])

        # ---- mm2 bf16 ----
        def mm2_bf16(h1, t0, csz):
            # hh-major so w2b half-tiles die as early as possible (their
            # slots host the fp8 w2 loads).
            for hh in range(2):
                src = w2b_t[hh]
                for ct in range(csz // P):
                    gt = t0 // P + ct
                    ps2 = p2.tile([P, 512], dt.float32, name="ps2")
                    for kf in range(KF):
                        nc.tensor.matmul(
                            ps2[:], h1[:, kf, ct * P:(ct + 1) * P], src[:, kf, :],
                            start=(kf == 0), stop=(kf == KF - 1))
                    ob = opool.tile([P, 512], dt.float32, name="ob")
                    nc.vector.tensor_scalar_mul(ob[:], ps2[:], wmb_s[:, gt:gt + 1])
                    nc.sync.dma_start(
                        yc[gt * P:(gt + 1) * P, hh * 512:(hh + 1) * 512], ob[:])

        # ---- mm2 fp8 (DoubleRow) ----
        def mm2_fp8(h1, w2t, wmt, t0, csz, fill=None):
            # fill: optional generator yielding thunks of extra PE work to
            # interleave between psum groups (keeps PE fed when the groups
            # alone would be drained faster than ACT/DVE latency allows).
            for tt in range(csz // 64):
                for hh in range(4):
                    ps2 = p2.tile([P, 512], dt.float32, name="ps2")[:64, :256]
                    for kp in range(KF // 2):
                        nc.tensor.matmul(
                            ps2[:], h1[:, 2 * kp:2 * kp + 2, tt * 64:(tt + 1) * 64],
                            w2t[:, 2 * kp:2 * kp + 2, hh * 256:(hh + 1) * 256],
                            start=(kp == 0), stop=(kp == KF // 2 - 1),
                            perf_mode=DR)
                    ob = opool.tile([P, 512], dt.float32, name="ob")[:64, :256]
                    nc.vector.tensor_scalar_mul(ob[:], ps2[:], wmt[:, tt:tt + 1])
                    nc.sync.dma_start(
                        yc[t0 + tt * 64:t0 + (tt + 1) * 64,
                           hh * 256:(hh + 1) * 256], ob[:])
                    if fill is not None:
                        for thunk in fill(tt * 4 + hh):
                            thunk()

        # ---- schedule ----
        h1 = h1bp.tile([P, KF, 512], dt.bfloat16, name="h1b")
        mm1(xc0, 512, w1b_q, b1o_s, h1)
        mm2_bf16(h1, 0, 512)

        h1 = h1bp.tile([P, KF, 512], dt.bfloat16, name="h1b")[:, :, :256]
        mm1(xc1, 256, w1b_q, b1o_s, h1)
        mm2_bf16(h1, 512, 256)

        h1a = h1fp.tile([P, KF, FA], dt.float8e4, name="h1f")
        mm1(xca, FA, w1b_q, b1o_s, h1a)

        # cB's tiny mm1 is ACT-latency-paced on its own (22ns PE bubbles
        # that also reset the p-state ramp); interleave its f-tiles
        # between cA-mm2 psum groups so PE stays saturated.
        h1b2 = h1fp.tile([P, KF, FA], dt.float8e4, name="h1f")[:, :, :FB]

        def emit_cb_ft(ft):
            src = w1s_q[ft // 4]
            ps = p1.tile([P, 512], dt.float32, name="ps1")[:, :FB]
            for k in range(KH):
                nc.tensor.matmul(
                    ps[:], src[:, k, (ft % 4) * P:(ft % 4 + 1) * P], xcb[:, k],
                    start=(k == 0), stop=(k == KH - 1))
            nc.scalar.activation(h1b2[:, ft], ps[:], AF.Gelu,
                                 bias=b1s_s[:, ft:ft + 1])

        def fill_cb(group_idx):
            ft0 = group_idx * 2
            return [(lambda ft=ft: emit_cb_ft(ft))
                    for ft in range(ft0, min(ft0 + 2, KF))]

        mm2_fp8(h1a, w2a_t, wma_s, G, FA, fill=fill_cb)
        mm2_fp8(h1b2, w2s_t, wms_s, G + FA, FB)
    return nc


def _get_nc(G, FA, FB):
    key = (G, FA, FB)
    if key not in _CACHE:
        nc = _build_nc(G, FA, FB)
        nc.finalize()
        _CACHE[key] = nc
    return _CACHE[key]


def dispatch(hidden_states, router_w, router_b):
    """Host router: top-2 ids + renormalized combine weights per token."""
    x = np.asarray(hidden_states, dtype=np.float32).reshape(T, H)
    logits = x @ np.asarray(router_w, dtype=np.float32)
    logits = logits + np.asarray(router_b, dtype=np.float32)
    part = np.argpartition(logits, E - 2, axis=1)[:, E - 2:]      # [T,2] unordered
    pv = np.take_along_axis(logits, part, axis=1)
    swap = pv[:, 0] > pv[:, 1]
    i1 = np.where(swap, part[:, 0], part[:, 1])
    i2 = np.where(swap, part[:, 1], part[:, 0])
    l1 = logits[np.arange(T), i1]
    l2 = logits[np.arange(T), i2]
    e2 = np.exp((l2 - l1).astype(np.float64))
    wt1 = (1.0 / (1.0 + e2)).astype(np.float32)
    wt2 = (e2 / (1.0 + e2)).astype(np.float32)
    return x, i1, i2, wt1, wt2


def plan(i1, i2, wt1, wt2, G, FA, FB):
    """Token->(core, group) assignment."""
    bf_tok, bf_wt, a_tok, a_wt, spill = [], [], [], [], []
    for m in range(E):
        tk = np.concatenate([np.where(i1 == m)[0], np.where(i2 == m)[0]])
        wt = np.concatenate([wt1[i1 == m], wt2[i2 == m]])
        o = np.argsort(-wt)
        tk, wt = tk[o], wt[o]
        bf_tok.append(tk[:G])
        bf_wt.append(wt[:G])
        a_tok.append(tk[G:G + FA])
        a_wt.append(wt[G:G + FA])
        rest_t, rest_w = tk[G + FA:], wt[G + FA:]
        for s0 in range(0, len(rest_t), FB):
            spill.append((m, rest_t[s0:s0 + FB], rest_w[s0:s0 + FB]))
    assert len(spill) <= E, f"spill slots {len(spill)} > {E}"
    while len(spill) < E:
        spill.append((0, np.zeros(0, np.int64), np.zeros(0, np.float32)))
    return bf_tok, bf_wt, a_tok, a_wt, spill


def make_in_maps(hidden_states, router_w, router_b, w1, b1, w2, b2,
                 G=G_DEF, FA=FA_DEF, FB=FB_DEF):
    x, i1, i2, wt1, wt2 = dispatch(hidden_states, router_w, router_b)
    bf_tok, bf_wt, a_tok, a_wt, spill = plan(i1, i2, wt1, wt2, G, FA, FB)
    w1 = np.asarray(w1, dtype=np.float32)
    w2 = np.asarray(w2, dtype=np.float32)
    b1 = np.asarray(b1, dtype=np.float32)
    b2 = np.asarray(b2, dtype=np.float32)
    xt = np.ascontiguousarray(x.T)                      # [H, T]
    GT = G // P

    def wcol(wts, cap, rows, scale):
        ncol = cap // rows
        out = np.zeros((rows, ncol), dtype=np.float32)
        wv = np.zeros(cap, dtype=np.float32)
        wv[:len(wts)] = wts * scale
        for c in range(ncol):
            out[:, c] = wv[c * rows:(c + 1) * rows]
        return out

    w1b16 = [np.ascontiguousarray(w1[m].astype(bf16)) for m in range(E)]
    w2f8 = [np.ascontiguousarray((w2[m] * WSCALE).astype(f8)) for m in range(E)]
    b1r = [np.ascontiguousarray(b1[m].reshape(KF, P).T) for m in range(E)]

    in_maps = []
    for m in range(E):
        sm, st, sw = spill[m]
        xbm = np.zeros((H, G + FA + FB), dtype=bf16)
        xbm[:, :len(bf_tok[m])] = xt[:, bf_tok[m]].astype(bf16)
        xbm[:, G:G + len(a_tok[m])] = xt[:, a_tok[m]].astype(bf16)
        xbm[:, G + FA:G + FA + len(st)] = xt[:, st].astype(bf16)
        in_maps.append({
            "xb": xbm,
            "w1b": w1b16[m],
            "w2b": np.ascontiguousarray(w2[m].astype(bf16)),
            "w1s": w1b16[sm],
            "w2a": w2f8[m], "w2s": w2f8[sm],
            "b1o": b1r[m], "b1s": b1r[sm],
            "wmb": wcol(bf_wt[m], GT * P, P, 1.0),
            "wma": wcol(a_wt[m], FA, 64, 1.0 / WSCALE),
            "wms": wcol(sw, FB, 64, 1.0 / WSCALE),
        })
    meta = (bf_tok, a_tok, spill, i1, i2, wt1, wt2)
    return in_maps, meta


def run_device(in_maps, G=G_DEF, FA=FA_DEF, FB=FB_DEF):
    from concourse.bass_utils import run_bass_kernel_spmd

    nc = _get_nc(G, FA, FB)
    res = run_bass_kernel_spmd(nc, in_maps, core_ids=list(range(E)))
    return res.results


def kernel(hidden_states, router_w, router_b, w1, b1, w2, b2):
    G, FA, FB = G_DEF, FA_DEF, FB_DEF
    in_maps, meta = make_in_maps(
        hidden_states, router_w, router_b, w1, b1, w2, b2, G, FA, FB)
    bf_tok, a_tok, spill, i1, i2, wt1, wt2 = meta
    b2 = np.asarray(b2, dtype=np.float32)
    # One retry guards against rare transient NRT/axon failures.
    last_err = None
    for attempt in range(3):
        try:
            results = run_device(in_maps, G, FA, FB)
        except Exception as e:
            last_err = e
            import time as _time
            _time.sleep(10)
            continue
        acc = np.zeros((T, H), dtype=np.float32)
        for m in range(E):
            ycm = np.asarray(results[m]["yc"], dtype=np.float32)
            acc[bf_tok[m]] += ycm[:len(bf_tok[m])]
            if len(a_tok[m]):
                acc[a_tok[m]] += ycm[G:G + len(a_tok[m])]
            sm, st, sw = spill[m]
            if len(st):
                acc[st] += ycm[G + FA:G + FA + len(st)]
        # b2 contribution (combine-weighted), host-side
        acc += wt1[:, None] * b2[i1] + wt2[:, None] * b2[i2]
        if np.isfinite(acc).all() and np.abs(acc).max() < 1e4:
            return acc.reshape(B, S, H)
    if last_err is not None:
        raise last_err
    return acc.reshape(B, S, H)


# revision 29
# speedup vs baseline: 1.1871x; 1.0196x over previous
"""MoE layer (8 experts, top-2) on 8 TRN2 NeuronCores, expert-parallel.

Strategy (sparse dispatch + mixed-precision mm2):
  - Host computes the router (fp32 logits, top-2, softmax renormalize) and
    dispatches: core m owns expert m's weights.
  - Per expert, tokens sort by combine weight. The G=768 highest-weight
    token-expert pairs run fully in bf16. The remainder (low combine
    weight) runs mm1 in bf16 but mm2 in fp8 e4m3 with DoubleRow perf mode
    (2x PE throughput); the fp8 quantization error is suppressed by those
    tokens' small combine weights (overall rel-err ~1.4e-2 < 2e-2).
  - SPMD static shapes per core: G bf16 tokens + 256-token fp8 slot A
    (own expert) + 64-token fp8 slot B (another expert's overflow, that
    expert's weights are DMA'd to this core). Underfilled slots pad with
    zero-weight tokens.
  - Combine weights apply on device (DVE); b2 is added on host (it only
    multiplies combine weights, which the host has).
"""

from contextlib import ExitStack

import ml_dtypes
import numpy as np

P = 128
B, S, H, F, E = 2, 2048, 1024, 4096, 8
T = B * S            # 4096 tokens
KH = H // P          # 8   k-subtiles over H
KF = F // P          # 32  k-subtiles over F

G_DEF = 640          # bf16 tokens per core (multiple of 128)
FA_DEF = 384         # fp8 slot A capacity (own expert, multiple of 64)
FB_DEF = 64          # fp8 slot B capacity (spill expert, multiple of 64)

bf16 = ml_dtypes.bfloat16
f8 = ml_dtypes.float8_e4m3fn
WSCALE = 64.0        # fp8 w2 pre-scale (folded into combine weights)

_CACHE = {}
WARMUP = 30


def _build_nc(G, FA, FB):
    import concourse.mybir as mybir
    import concourse.tile as tile
    from concourse import bacc

    dt = mybir.dt
    AF = mybir.ActivationFunctionType
    DR = mybir.MatmulPerfMode.DoubleRow

    GT = G // P                    # bf16 token tiles (6)
    NT = G + FA + FB               # tokens per core (1088)
    NAT = FA // 64                 # fp8 A mm2 tiles (4)
    NBT = FB // 64                 # fp8 B mm2 tiles (1)

    nc = bacc.Bacc(
        "TRN2", target_bir_lowering=False, debug=False, num_devices=E)

    xb = nc.declare_dram_parameter("xb", [H, NT], dt.bfloat16, isOutput=False)
    w1b = nc.declare_dram_parameter("w1b", [H, F], dt.bfloat16, isOutput=False)
    w2b = nc.declare_dram_parameter("w2b", [F, H], dt.bfloat16, isOutput=False)
    w1s = nc.declare_dram_parameter("w1s", [H, F], dt.bfloat16, isOutput=False)
    w2a = nc.declare_dram_parameter("w2a", [F, H], dt.float8e4, isOutput=False)
    w2s = nc.declare_dram_parameter("w2s", [F, H], dt.float8e4, isOutput=False)
    b1o = nc.declare_dram_parameter("b1o", [P, KF], dt.float32, isOutput=False)
    b1s = nc.declare_dram_parameter("b1s", [P, KF], dt.float32, isOutput=False)
    wmb = nc.declare_dram_parameter("wmb", [P, GT], dt.float32, isOutput=False)
    wma = nc.declare_dram_parameter("wma", [64, NAT], dt.float32, isOutput=False)
    wms = nc.declare_dram_parameter("wms", [64, NBT], dt.float32, isOutput=False)
    yc = nc.declare_dram_parameter("yc", [NT, H], dt.float32, isOutput=True)

    xb_r = xb.rearrange("(k p) t -> p k t", p=P)
    w1b_r = w1b.rearrange("(k p) f -> p k f", p=P)
    w2b_r = w2b.rearrange("(k p) h -> p k h", p=P)
    w1s_r = w1s.rearrange("(k p) f -> p k f", p=P)
    w2a_r = w2a.rearrange("(k p) h -> p k h", p=P)
    w2s_r = w2s.rearrange("(k p) h -> p k h", p=P)

    with ExitStack() as ctx:
        tc = ctx.enter_context(tile.TileContext(nc))
        const = ctx.enter_context(tc.tile_pool(name="const", bufs=1))
        w1pool = ctx.enter_context(tc.tile_pool(name="w1", bufs=8))
        w2pool = ctx.enter_context(tc.tile_pool(name="w2", bufs=2))
        xbp = ctx.enter_context(tc.tile_pool(name="xb", bufs=2))
        h1bp = ctx.enter_context(tc.tile_pool(name="h1b", bufs=1))
        h1fp = ctx.enter_context(tc.tile_pool(name="h1f", bufs=1))
        h1fbp = ctx.enter_context(tc.tile_pool(name="h1fb", bufs=1))
        opool = ctx.enter_context(tc.tile_pool(name="ob", bufs=6))
        p1 = ctx.enter_context(tc.tile_pool(name="p1", bufs=5, space="PSUM"))
        p2 = ctx.enter_context(tc.tile_pool(name="p2", bufs=3, space="PSUM"))

        # ---- PE warmup ----
        # The cost model ramps PE 0.65->1.2->2.4 GHz over the first 3us of
        # continuous execution. Fill the initial DMA wait (~6us) with dummy
        # matmuls on memset data so real matmuls start at full clock.
        wrm = const.tile([P, 640], dt.bfloat16)
        nc.vector.memset(wrm[:], 0.0)
        for i in range(WARMUP):
            pw = p1.tile([P, 512], dt.float32, name="ps1")[:, :256]
            nc.tensor.matmul(pw[:], wrm[:, :P], wrm[:, P:P + 256],
                             start=True, stop=True)

        C0 = G - 256                   # bf16 chunk sizes (multiples of 128)
        C1 = 256

        # ---- x chunk loads ----
        def load_x(t0, csz):
            xt = xbp.tile([P, KH, 512], dt.bfloat16, name="xbt")[:, :, :csz]
            for k in range(KH):
                nc.sync.dma_start(xt[:, k], xb_r[:, k, t0:t0 + csz])
            return xt

        # Startup order: x chunk-0's k=0 plane, then w1b's first f-tile
        # (small, so the first real matmul is runnable after ~0.4MB), then
        # b1o for the first activation, then the rest.
        xc0 = xbp.tile([P, KH, 512], dt.bfloat16, name="xbt")[:, :, :C0]
        nc.sync.dma_start(xc0[:, 0], xb_r[:, 0, 0:C0])
        w1h0 = const.tile([P, KH, 128], dt.bfloat16)
        nc.sync.dma_start(w1h0[:], w1b_r[:, :, 0:128])
        b1o_s = const.tile([P, KF], dt.float32)
        nc.sync.dma_start(b1o_s[:], b1o[:])
        for k in range(1, KH):
            nc.sync.dma_start(xc0[:, k], xb_r[:, k, 0:C0])
        w1h1 = const.tile([P, KH, 384], dt.bfloat16)
        nc.sync.dma_start(w1h1[:], w1b_r[:, :, 128:512])
        b1s_s = const.tile([P, KF], dt.float32)
        nc.sync.dma_start(b1s_s[:], b1s[:])
        wmb_s = const.tile([P, GT], dt.float32)
        nc.sync.dma_start(wmb_s[:], wmb[:])
        wma_s = const.tile([64, NAT], dt.float32)
        nc.sync.dma_start(wma_s[:], wma[:])
        wms_s = const.tile([64, NBT], dt.float32)
        nc.sync.dma_start(wms_s[:], wms[:])

        # w1 weights: 1MB tiles so matmuls march at 1MB granularity.
        # w1b covers f 512:4096 in q1..q7 (f 0:512 lives in w1h0/w1h1).
        w1b_q = []
        for q in range(1, 8):
            t = w1pool.tile([P, KH, 512], dt.bfloat16, name="w1")
            w1b_q.append(t)
            nc.sync.dma_start(t[:], w1b_r[:, :, q * 512:(q + 1) * 512])
        w2b_t = []
        for half in range(2):
            t = w2pool.tile([P, KF, H // 2], dt.bfloat16, name="w2")
            w2b_t.append(t)
            for hc in range(2):
                nc.sync.dma_start(
                    t[:, :, hc * 256:(hc + 1) * 256],
                    w2b_r[:, :, half * (H // 2) + hc * 256:half * (H // 2) + (hc + 1) * 256])

        # Remaining x chunks: issued ahead of the slot-blocked weight DMAs
        # so they don't queue behind them. Pool rotation: xc1 -> slot 1,
        # xca -> slot 0 (waits c0-mm1), xcb -> slot 1 (waits c1-mm1).
        xc1 = load_x(C0, C1)
        xca = load_x(G, FA)
        xcb = load_x(G + FA, FB)

        # ---- late weights; slot reuse + FIFO order hides their DMA ----
        # w2a reuses w2b half1's slot (dead ~27us before bf16 end), w1s
        # chunks reuse w1b's slots (dead progressively during cA-mm1),
        # w2s reuses w2b half2's slot; issue order matches need order.
        w2a_t = w2pool.tile([P, KF, H], dt.float8e4, name="w2")
        for hc in range(2):
            nc.sync.dma_start(w2a_t[:, :, hc * 512:(hc + 1) * 512],
                              w2a_r[:, :, hc * 512:(hc + 1) * 512])
        w2s_t = w2pool.tile([P, KF, H], dt.float8e4, name="w2")
        for hc in range(2):
            nc.sync.dma_start(w2s_t[:, :, hc * 512:(hc + 1) * 512],
                              w2s_r[:, :, hc * 512:(hc + 1) * 512])
        w1s_q = []
        for q in range(8):
            t = w1pool.tile([P, KH, 512], dt.bfloat16, name="w1")
            w1s_q.append(t)
            nc.sync.dma_start(t[:], w1s_r[:, :, q * 512:(q + 1) * 512])

        # ---- mm1 (bf16) ----
        def w1b_src(ft):
            if ft == 0:
                return w1h0, 0
            if ft < 4:
                return w1h1, (ft - 1) * P
            return w1b_q[ft // 4 - 1], (ft % 4) * P

        def w1s_src(ft):
            return w1s_q[ft // 4], (ft % 4) * P

        def mm1(xt, csz, src_fn, b1t, h1):
            for ft in range(KF):
                src, c0 = src_fn(ft)
                ps = p1.tile([P, 512], dt.float32, name="ps1")[:, :csz]
                for k in range(KH):
                    nc.tensor.matmul(
                        ps[:], src[:, k, c0:c0 + P], xt[:, k],
                        start=(k == 0), stop=(k == KH - 1))
                nc.scalar.activation(h1[:, ft], ps[:], AF.Gelu,
                                     bias=b1t[:, ft:ft + 1])

        # ---- mm2 bf16 ----
        def mm2_bf16(h1, t0, csz):
            # hh-major so w2b half-tiles die as early as possible (their
            # slots host the fp8 w2 loads).
            for hh in range(2):
                src = w2b_t[hh]
                for ct in range(csz // P):
                    gt = t0 // P + ct
                    ps2 = p2.tile([P, 512], dt.float32, name="ps2")
                    for kf in range(KF):
                        nc.tensor.matmul(
                            ps2[:], h1[:, kf, ct * P:(ct + 1) * P], src[:, kf, :],
                            start=(kf == 0), stop=(kf == KF - 1))
                    ob = opool.tile([P, 512], dt.float32, name="ob")
                    nc.vector.tensor_scalar_mul(ob[:], ps2[:], wmb_s[:, gt:gt + 1])
                    nc.sync.dma_start(
                        yc[gt * P:(gt + 1) * P, hh * 512:(hh + 1) * 512], ob[:])

        # ---- mm2 fp8 (DoubleRow) ----
        def mm2_fp8(h1, w2t, wmt, t0, csz, fill=None, tail_split=False):
            # fill: optional generator yielding thunks of extra PE work to
            # interleave between psum groups (keeps PE fed when the groups
            # alone would be drained faster than ACT/DVE latency allows).
            # tail_split: break the very last psum group into column halves
            # so its DVE+DMA drain overlaps the second half's matmuls.
            for tt in range(csz // 64):
                for hh in range(4):
                    last = tail_split and tt == csz // 64 - 1 and hh == 3
                    parts = ((0, 128), (128, 64), (192, 64)) if last else ((0, 256),)
                    for (o0, wid) in parts:
                        ps2 = p2.tile([P, 512], dt.float32, name="ps2")[:64, :wid]
                        for kp in range(KF // 2):
                            nc.tensor.matmul(
                                ps2[:], h1[:, 2 * kp:2 * kp + 2, tt * 64:(tt + 1) * 64],
                                w2t[:, 2 * kp:2 * kp + 2,
                                    hh * 256 + o0:hh * 256 + o0 + wid],
                                start=(kp == 0), stop=(kp == KF // 2 - 1),
                                perf_mode=DR)
                        ob = opool.tile([P, 512], dt.float32, name="ob")[:64, :wid]
                        nc.vector.tensor_scalar_mul(ob[:], ps2[:], wmt[:, tt:tt + 1])
                        nc.sync.dma_start(
                            yc[t0 + tt * 64:t0 + (tt + 1) * 64,
                               hh * 256 + o0:hh * 256 + o0 + wid], ob[:])
                    if fill is not None:
                        for thunk in fill(tt * 4 + hh):
                            thunk()

        # ---- schedule ----
        h1 = h1bp.tile([P, KF, C0], dt.bfloat16, name="h1b")
        mm1(xc0, C0, w1b_src, b1o_s, h1)
        mm2_bf16(h1, 0, C0)

        h1 = h1bp.tile([P, KF, C0], dt.bfloat16, name="h1b")[:, :, :C1]
        mm1(xc1, C1, w1b_src, b1o_s, h1)
        mm2_bf16(h1, C0, C1)

        h1a = h1fp.tile([P, KF, FA], dt.float8e4, name="h1f")
        mm1(xca, FA, w1b_src, b1o_s, h1a)

        # cB's tiny mm1 is ACT-latency-paced on its own (22ns PE bubbles
        # that also reset the p-state ramp); interleave its f-tiles
        # between cA-mm2 psum groups so PE stays saturated.
        h1b2 = h1fbp.tile([P, KF, FB], dt.float8e4, name="h1fb")

        def emit_cb_ft(ft):
            src, c0 = w1s_src(ft)
            ps = p1.tile([P, 512], dt.float32, name="ps1")[:, :FB]
            for k in range(KH):
                nc.tensor.matmul(
                    ps[:], src[:, k, c0:c0 + P], xcb[:, k],
                    start=(k == 0), stop=(k == KH - 1))
            nc.scalar.activation(h1b2[:, ft], ps[:], AF.Gelu,
                                 bias=b1s_s[:, ft:ft + 1])

        def fill_cb(group_idx):
            ft0 = group_idx * 2
            return [(lambda ft=ft: emit_cb_ft(ft))
                    for ft in range(ft0, min(ft0 + 2, KF))]

        mm2_fp8(h1a, w2a_t, wma_s, G, FA, fill=fill_cb)
        mm2_fp8(h1b2, w2s_t, wms_s, G + FA, FB, tail_split=True)
    return nc


def _get_nc(G, FA, FB):
    key = (G, FA, FB)
    if key not in _CACHE:
        nc = _build_nc(G, FA, FB)
        nc.finalize()
        _CACHE[key] = nc
    return _CACHE[key]


def dispatch(hidden_states, router_w, router_b):
    """Host router: top-2 ids + renormalized combine weights per token."""
    x = np.asarray(hidden_states, dtype=np.float32).reshape(T, H)
    logits = x @ np.asarray(router_w, dtype=np.float32)
    logits = logits + np.asarray(router_b, dtype=np.float32)
    part = np.argpartition(logits, E - 2, axis=1)[:, E - 2:]      # [T,2] unordered
    pv = np.take_along_axis(logits, part, axis=1)
    swap = pv[:, 0] > pv[:, 1]
    i1 = np.where(swap, part[:, 0], part[:, 1])
    i2 = np.where(swap, part[:, 1], part[:, 0])
    l1 = logits[np.arange(T), i1]
    l2 = logits[np.arange(T), i2]
    e2 = np.exp((l2 - l1).astype(np.float64))
    wt1 = (1.0 / (1.0 + e2)).astype(np.float32)
    wt2 = (e2 / (1.0 + e2)).astype(np.float32)
    return x, i1, i2, wt1, wt2


def plan(i1, i2, wt1, wt2, G, FA, FB):
    """Token->(core, group) assignment."""
    bf_tok, bf_wt, a_tok, a_wt, spill = [], [], [], [], []
    for m in range(E):
        tk = np.concatenate([np.where(i1 == m)[0], np.where(i2 == m)[0]])
        wt = np.concatenate([wt1[i1 == m], wt2[i2 == m]])
        o = np.argsort(-wt)
        tk, wt = tk[o], wt[o]
        bf_tok.append(tk[:G])
        bf_wt.append(wt[:G])
        a_tok.append(tk[G:G + FA])
        a_wt.append(wt[G:G + FA])
        rest_t, rest_w = tk[G + FA:], wt[G + FA:]
        for s0 in range(0, len(rest_t), FB):
            spill.append((m, rest_t[s0:s0 + FB], rest_w[s0:s0 + FB]))
    assert len(spill) <= E, f"spill slots {len(spill)} > {E}"
    while len(spill) < E:
        spill.append((0, np.zeros(0, np.int64), np.zeros(0, np.float32)))
    return bf_tok, bf_wt, a_tok, a_wt, spill


def make_in_maps(hidden_states, router_w, router_b, w1, b1, w2, b2,
                 G=G_DEF, FA=FA_DEF, FB=FB_DEF):
    x, i1, i2, wt1, wt2 = dispatch(hidden_states, router_w, router_b)
    bf_tok, bf_wt, a_tok, a_wt, spill = plan(i1, i2, wt1, wt2, G, FA, FB)
    w1 = np.asarray(w1, dtype=np.float32)
    w2 = np.asarray(w2, dtype=np.float32)
    b1 = np.asarray(b1, dtype=np.float32)
    b2 = np.asarray(b2, dtype=np.float32)
    xt = np.ascontiguousarray(x.T)                      # [H, T]
    GT = G // P

    def wcol(wts, cap, rows, scale):
        ncol = cap // rows
        out = np.zeros((rows, ncol), dtype=np.float32)
        wv = np.zeros(cap, dtype=np.float32)
        wv[:len(wts)] = wts * scale
        for c in range(ncol):
            out[:, c] = wv[c * rows:(c + 1) * rows]
        return out

    w1b16 = [np.ascontiguousarray(w1[m].astype(bf16)) for m in range(E)]
    w2f8 = [np.ascontiguousarray((w2[m] * WSCALE).astype(f8)) for m in range(E)]
    b1r = [np.ascontiguousarray(b1[m].reshape(KF, P).T) for m in range(E)]

    in_maps = []
    for m in range(E):
        sm, st, sw = spill[m]
        xbm = np.zeros((H, G + FA + FB), dtype=bf16)
        xbm[:, :len(bf_tok[m])] = xt[:, bf_tok[m]].astype(bf16)
        xbm[:, G:G + len(a_tok[m])] = xt[:, a_tok[m]].astype(bf16)
        xbm[:, G + FA:G + FA + len(st)] = xt[:, st].astype(bf16)
        in_maps.append({
            "xb": xbm,
            "w1b": w1b16[m],
            "w2b": np.ascontiguousarray(w2[m].astype(bf16)),
            "w1s": w1b16[sm],
            "w2a": w2f8[m], "w2s": w2f8[sm],
            "b1o": b1r[m], "b1s": b1r[sm],
            "wmb": wcol(bf_wt[m], GT * P, P, 1.0),
            "wma": wcol(a_wt[m], FA, 64, 1.0 / WSCALE),
            "wms": wcol(sw, FB, 64, 1.0 / WSCALE),
        })
    meta = (bf_tok, a_tok, spill, i1, i2, wt1, wt2)
    return in_maps, meta


def run_device(in_maps, G=G_DEF, FA=FA_DEF, FB=FB_DEF):
    from concourse.bass_utils import run_bass_kernel_spmd

    nc = _get_nc(G, FA, FB)
    res = run_bass_kernel_spmd(nc, in_maps, core_ids=list(range(E)))
    return res.results


def kernel(hidden_states, router_w, router_b, w1, b1, w2, b2):
    G, FA, FB = G_DEF, FA_DEF, FB_DEF
    in_maps, meta = make_in_maps(
        hidden_states, router_w, router_b, w1, b1, w2, b2, G, FA, FB)
    bf_tok, a_tok, spill, i1, i2, wt1, wt2 = meta
    b2 = np.asarray(b2, dtype=np.float32)
    # One retry guards against rare transient NRT/axon failures.
    last_err = None
    for attempt in range(3):
        try:
            results = run_device(in_maps, G, FA, FB)
        except Exception as e:
            last_err = e
            import time as _time
            _time.sleep(10)
            continue
        acc = np.zeros((T, H), dtype=np.float32)
        for m in range(E):
            ycm = np.asarray(results[m]["yc"], dtype=np.float32)
            acc[bf_tok[m]] += ycm[:len(bf_tok[m])]
            if len(a_tok[m]):
                acc[a_tok[m]] += ycm[G:G + len(a_tok[m])]
            sm, st, sw = spill[m]
            if len(st):
                acc[st] += ycm[G + FA:G + FA + len(st)]
        # b2 contribution (combine-weighted), host-side
        acc += wt1[:, None] * b2[i1] + wt2[:, None] * b2[i2]
        if np.isfinite(acc).all() and np.abs(acc).max() < 1e4:
            return acc.reshape(B, S, H)
    if last_err is not None:
        raise last_err
    return acc.reshape(B, S, H)
